# revision 14
# baseline (speedup 1.0000x reference)
"""BitNet GQA attention layer on 8 TRN2 NeuronCores — v2.

Sharding: token-parallel with zigzag causal balance. B*S = 2048 tokens ->
256 per core. Core c (batch b=c//4, zig z=c%4) owns query tiles {z, 7-z}
of batch b, so every core computes the same 12 score blocks per head
(4 for tile z masked per-core, 8 for tile 7-z) instead of 20.

Weights are split 8-way along the contraction dim for quantization (one
tiny AllReduce for all four matrices' abs-sums), then ternary fp8 weights
are AllGathered (k+v merged, then q, then o last). K/V activations are
AllGathered within each batch's 4-core group (k and v merged into one
collective). All transposes (x, q, k, attn) run on the DMA XBAR
(bf16 SBUF->SBUF), none on the PE. BitNet matmuls are exact integer
arithmetic in bf16 x fp8 with fp32 PSUM accumulation; projection loops
run hi-outer/chunk-inner so one LDWEIGHTS serves 4-8 matmuls.
"""

import sys

sys.path.insert(0, "/opt/trn_rl_repo")

import numpy as np
import ml_dtypes

import concourse.bass as bass
import concourse.mybir as mybir
import concourse.tile as tile
from concourse import bacc
from concourse import bass_utils

F32 = mybir.dt.float32
BF16 = mybir.dt.bfloat16
FP8 = mybir.dt.float8e4
AX = mybir.AxisListType.X
OP = mybir.AluOpType
AF = mybir.ActivationFunctionType

B, S, H = 2, 1024, 2048
NH, NKV, HD = 16, 8, 128
NC = 8
T = (B * S) // NC  # 256 tokens per core
TB = T // 128  # 2 token tiles per core
HSL = H // NC  # 256 weight rows per core
EPS = 1e-6
RND = 12582912.0  # 1.5 * 2**23: fp32 add => round-to-nearest-even
INV_SQRT_HD = 1.0 / float(np.sqrt(HD))
KTILES = S // 128  # 8 key tiles per batch
GROUP = 4  # cores per batch
NJ0 = 4  # key slots computed for q-tile0 (covers z <= 3)
NJ1 = KTILES  # key slots computed for q-tile1

OQ, OK, OV, OO = H, NKV * HD, NKV * HD, H  # 2048, 1024, 1024, 2048
OW = {"q": OQ, "k": OK, "v": OV, "o": OO}
WNUMEL = {m: OW[m] * H for m in OW}
HI_N = H // 128  # 16 contraction tiles

_CACHE = {}


def _build():
    nc = bacc.Bacc("TRN2", target_bir_lowering=False, debug=False, num_devices=NC)

    x_sl = nc.dram_tensor("x_sl", [T, H], F32, kind="ExternalInput")
    cosq = nc.dram_tensor("cosq", [T, HD], F32, kind="ExternalInput")
    sinq = nc.dram_tensor("sinq", [T, HD], F32, kind="ExternalInput")
    cosk = nc.dram_tensor("cosk", [T, HD], F32, kind="ExternalInput")
    sink = nc.dram_tensor("sink", [T, HD], F32, kind="ExternalInput")
    w_sl = {
        "q": nc.dram_tensor("wq_sl", [HSL, OQ], F32, kind="ExternalInput"),
        "k": nc.dram_tensor("wk_sl", [HSL, OK], F32, kind="ExternalInput"),
        "v": nc.dram_tensor("wv_sl", [HSL, OV], F32, kind="ExternalInput"),
        "o": nc.dram_tensor("wo_sl", [HSL, OO], F32, kind="ExternalInput"),
    }
    # mask[p, a, j, f]: a=0 -> pel[k=p, slot j (abs key tile j), q=f] of tile z
    #                  a=1 -> slots 4..7 (abs key tiles 4..7) of tile 7-z
    mask_in = nc.dram_tensor("mask", [128, 2, NJ0, 128], BF16, kind="ExternalInput")
    # cols 0-3: numel for k,v,q,o ; cols 4-7: 1/numel for k,v,q,o
    wconst = nc.dram_tensor("wconst", [1, 8], F32, kind="ExternalInput")
    out = nc.dram_tensor("out", [T, H], F32, kind="ExternalOutput")

    with tile.TileContext(nc) as tc:
        _build_body(nc, tc, x_sl, cosq, sinq, cosk, sink, w_sl, mask_in, wconst, out)

    nc.compile()
    return nc


def _build_body(nc, tc, x_sl, cosq, sinq, cosk, sink, w_sl, mask_in, wconst, out):
    sync = nc.sync

    with (
        tc.tile_pool(name="dram", bufs=1, space="DRAM") as dram,
        tc.tile_pool(name="const", bufs=1) as constp,
        tc.tile_pool(name="vecs", bufs=1) as vecs,
        tc.tile_pool(name="persist", bufs=1) as persist,
    ):
        # ---- DRAM bounce buffers for collectives ----
        wag_kv = dram.tile([HSL, OK + OV], FP8)
        wint_kv = dram.tile([H, OK + OV], FP8, addr_space="Shared")
        wag_q = dram.tile([HSL, OQ], FP8)
        wint_q = dram.tile([H, OQ], FP8, addr_space="Shared")
        wag_o = dram.tile([HSL, OO], FP8)
        wint_o = dram.tile([H, OO], FP8, addr_space="Shared")
        ar_in = dram.tile([1, 8], F32)
        ar_out = dram.tile([1, 8], F32, addr_space="Shared")
        # cols 0:2048 = kT (hk,t); cols 2048:4096 = v (a,hk,d)
        kv_in = dram.tile([128, NKV * T + TB * NKV * HD], BF16)
        kv_out = dram.tile([512, NKV * T + TB * NKV * HD], BF16)

        # ---- constants ----
        ones1 = constp.tile([1, 128], F32)
        nc.vector.memset(ones1, 1.0)
        onescol = constp.tile([128, 1], F32)
        nc.vector.memset(onescol, 1.0)
        wconst_sb = constp.tile([1, 8], F32)
        sync.dma_start(wconst_sb, wconst.ap())
        negrnd = constp.tile([128, 1], F32)
        nc.vector.memset(negrnd, -RND)
        epsb = constp.tile([128, 1], F32)
        nc.vector.memset(epsb, EPS)
        mask_sb = constp.tile([128, 2, NJ0, 128], BF16)
        sync.dma_start(mask_sb, mask_in.ap())
        cs = {}
        for nm, t in (("cq", cosq), ("sq", sinq), ("ck", cosk), ("sk", sink)):
            c = constp.tile([128, TB, HD], F32, name=f"cs_{nm}")
            sync.dma_start(c, t.ap().rearrange("(a p) d -> p a d", p=128))
            cs[nm] = c

        # persistent activations
        xqT = persist.tile([128, HI_N, T], BF16)  # [h%128, h//128, t]
        qT = persist.tile([128, NH, T], BF16)  # [d, head, t]
        kT = persist.tile([128, NKV, T], BF16)  # [d, kv head, t] (local)
        v_loc = persist.tile([128, TB, NKV, HD], BF16)
        q_sb = persist.tile([128, TB, OQ], F32)
        k_sb = persist.tile([128, TB, OK], F32)
        attn = persist.tile([128, TB, H], F32)
        aT = persist.tile([128, HI_N, T], BF16)
        # gathered k/v keyed by (group-core z, slot a); key tile j = (zj, aj)
        kT_all = persist.tile([128, NKV, GROUP, TB, 128], BF16)
        v_all = persist.tile([128, GROUP, TB, NKV, 130], BF16)

        # ====== Phase W: weight scales + quantize + pipelined allgathers ======
        dqx = []
        with (
            tc.tile_pool(name="wraw_k", bufs=1) as wraw_k,
            tc.tile_pool(name="wraw_v", bufs=1) as wraw_v,
            tc.tile_pool(name="wraw_q", bufs=1) as wraw_q,
            tc.tile_pool(name="wraw_o", bufs=1) as wraw_o,
            tc.tile_pool(name="wtmp", bufs=1) as wtmp,
            tc.tile_pool(name="wq8", bufs=2) as wq8,
            tc.tile_pool(name="xraw", bufs=1) as xraw,
            tc.tile_pool(name="psmall", bufs=2, space="PSUM") as psmall,
        ):
            wraws = {"k": wraw_k, "v": wraw_v, "q": wraw_q, "o": wraw_o}
            worder = ("k", "v", "q", "o")
            wab = {}
            for m in worder:
                for pt in range(2):
                    wr = wraws[m].tile([128, OW[m]], F32, name=f"wr_{m}{pt}")
                    sync.dma_start(wr, w_sl[m].ap()[pt * 128 : (pt + 1) * 128, :])
                    wab[(m, pt)] = wr
            # abs-sums: k,v on DVE; q,o on GpSimd (parallel)
            red0 = vecs.tile([128, 4], F32, name="red0")
            red1 = vecs.tile([128, 4], F32, name="red1")
            for mi, m in enumerate(worder):
                for pt, red in ((0, red0), (1, red1)):
                    nc.vector.tensor_reduce(
                        red[:, mi : mi + 1], wab[(m, pt)], axis=AX, op=OP.add,
                        apply_absolute_value=True,
                    )
            redc = vecs.tile([128, 4], F32, name="redc")
            nc.vector.tensor_add(redc, red0, red1)
            ps = psmall.tile([1, 4], F32, name="ps_sums", tag="psm")
            nc.tensor.matmul(ps, onescol, redc, start=True, stop=True)
            sums = vecs.tile([1, 8], F32, name="sums")
            nc.vector.memset(sums, 0.0)
            nc.scalar.copy(sums[:, 0:4], ps)
            nc.scalar.dma_start(ar_in, sums)
            nc.gpsimd.collective_compute(
                "AllReduce", OP.add, replica_groups=[list(range(NC))],
                ins=[ar_in.opt()], outs=[ar_out.opt()],
            )

            # ---- x: load + act_quant + XBAR transpose (fills the AR window) ----
            for tb in range(TB):
                xs = xraw.tile([128, H], F32, tag="xs", name=f"xs{tb}")
                sync.dma_start(xs, x_sl.ap()[tb * 128 : (tb + 1) * 128, :])
                axm = vecs.tile([128, 1], F32, name=f"axm{tb}")
                nc.vector.tensor_reduce(
                    axm, xs, axis=AX, op=OP.max, apply_absolute_value=True
                )
                rsx = vecs.tile([128, 1], F32, name=f"rsx{tb}")
                nc.vector.reciprocal(rsx, axm)
                sxq = vecs.tile([128, 1], F32, name=f"sxq{tb}")
                nc.vector.tensor_scalar_mul(sxq, rsx, 127.0)
                dq = vecs.tile([128, 1], F32, name=f"dqx{tb}")
                nc.vector.tensor_scalar_mul(dq, axm, 1.0 / 127.0)
                dqx.append(dq)
                nc.vector.tensor_scalar(
                    xs, xs, sxq, RND, op0=OP.mult, op1=OP.add
                )
                xqb = xraw.tile([128, H], BF16, tag="xqb", name=f"xqb{tb}")
                nc.scalar.activation(xqb, xs, AF.Identity, bias=negrnd)
                sync.dma_start(
                    xqT[:, :, tb * 128 : (tb + 1) * 128], xqb, transpose=True
                )

            # ---- scales from the AllReduce ----
            g = vecs.tile([1, 8], F32, name="g")
            nc.scalar.dma_start(g, ar_out)
            r4 = vecs.tile([1, 4], F32, name="r4")
            nc.vector.reciprocal(r4, g[:, 0:4])
            sw8 = vecs.tile([1, 8], F32, name="sw8")
            nc.vector.tensor_mul(sw8[:, 0:4], r4, wconst_sb[:, 0:4])
            nc.vector.tensor_mul(sw8[:, 4:8], g[:, 0:4], wconst_sb[:, 4:8])
            pb = psmall.tile([128, 8], F32, name="pb", tag="psm")
            nc.tensor.matmul(pb, ones1, sw8, start=True, stop=True)
            sb8 = vecs.tile([128, 8], F32, name="sb8")
            nc.scalar.copy(sb8, pb)
            rswb = {m: sb8[:, 4 + mi : 5 + mi] for mi, m in enumerate(worder)}

            def w_quant(m, mi, dst, col0):
                for pt in range(2):
                    wr = wab[(m, pt)]
                    t1 = wtmp.tile([128, OW[m]], F32, tag="wtmp", name=f"t1_{m}{pt}")
                    nc.vector.tensor_scalar(
                        t1, wr, sb8[:, mi : mi + 1], RND, op0=OP.mult, op1=OP.add
                    )
                    nc.scalar.activation(t1, t1, AF.Identity, bias=negrnd)
                    wi = wq8.tile([128, OW[m]], FP8, tag="wi")
                    nc.vector.tensor_scalar(wi, t1, 1.0, -1.0, op0=OP.min, op1=OP.max)
                    nc.scalar.dma_start(
                        dst[pt * 128 : (pt + 1) * 128, col0 : col0 + OW[m]], wi
                    )

            w_quant("k", 0, wag_kv, 0)
            w_quant("v", 1, wag_kv, OK)
            nc.gpsimd.collective_compute(
                "AllGather", OP.bypass, replica_groups=[list(range(NC))],
                ins=[wag_kv.opt()], outs=[wint_kv.opt()],
            )
            w_quant("q", 2, wag_q, 0)
            nc.gpsimd.collective_compute(
                "AllGather", OP.bypass, replica_groups=[list(range(NC))],
                ins=[wag_q.opt()], outs=[wint_q.opt()],
            )
            w_quant("o", 3, wag_o, 0)

        # dequant vectors (absmax/127 * 1/s_w)
        dqv = {}
        for m in ("q", "k", "v", "o"):
            for tb in range(TB):
                d = vecs.tile([128, 1], F32, name=f"dqv_{m}{tb}")
                nc.vector.tensor_mul(d, dqx[tb], rswb[m])
                dqv[(m, tb)] = d

        def rope_batch(src_sb, tb, nh, cosn, sinn, dstT, ropep, label):
            w = nh * 128
            blk = src_sb[:, tb, :]  # [128, w] f32
            sq = ropep.tile([128, w], F32, tag="unf", padded_shape=[128, NH * 128])
            nc.scalar.activation(sq, blk, AF.Square)
            ms = vecs.tile([128, nh], F32, name=f"ms_{label}{tb}")
            nc.vector.tensor_reduce(
                ms, sq.rearrange("p (h d) -> p h d", h=nh), axis=AX, op=OP.add
            )
            rms = vecs.tile([128, nh], F32, name=f"rms_{label}{tb}")
            nc.scalar.activation(rms, ms, AF.Sqrt, scale=1.0 / HD, bias=epsb)
            rn = vecs.tile([128, nh], F32, name=f"rn_{label}{tb}")
            nc.vector.reciprocal(rn, rms)
            rnb = rn.to_broadcast([128, nh, 128])
            blk3 = blk.rearrange("p (h d) -> p h d", h=nh)
            un2 = ropep.tile(
                [128, nh * 128], F32, tag="unf", padded_shape=[128, NH * 128],
                name="un2",
            )
            un = un2.rearrange("p (h d) -> p h d", h=nh)
            nc.vector.tensor_mul(un, blk3, rnb)
            cosb = (
                cs[cosn][:, tb, :]
                .rearrange("p (one d) -> p one d", one=1)
                .to_broadcast([128, nh, 128])
            )
            sinb = (
                cs[sinn][:, tb, :]
                .rearrange("p (one d) -> p one d", one=1)
                .to_broadcast([128, nh, 128])
            )
            ra2 = ropep.tile([128, nh * 128], F32, tag="ra", padded_shape=[128, NH * 128])
            ra = ra2.rearrange("p (h d) -> p h d", h=nh)
            nc.vector.tensor_mul(ra, un, cosb)
            rb2 = ropep.tile([128, nh * 128], F32, tag="rb", padded_shape=[128, NH * 128])
            rb = rb2.rearrange("p (h d) -> p h d", h=nh)
            nc.vector.tensor_mul(rb[:, :, 0:64], un[:, :, 64:128], sinb[:, :, 0:64])
            nc.vector.tensor_mul(rb[:, :, 64:128], un[:, :, 0:64], sinb[:, :, 64:128])
            raf = ropep.tile(
                [128, nh * 128], BF16, tag="raf", padded_shape=[128, NH * 128]
            )
            nc.vector.tensor_add(raf, ra2, rb2)
            sync.dma_start(
                dstT[:, 0:nh, tb * 128 : (tb + 1) * 128], raf, transpose=True
            )

        # ====== Phase KV/Q: projections (hi-outer, chunk-inner) ======
        def proj_load(wint_src, o_w, m, wpool):
            src3 = wint_src.rearrange("(hi p) o -> p hi o", p=128)
            chunks = []
            for cg in range(4):
                wst = wpool.tile(
                    [128, 4, o_w], FP8, tag="wst",
                    padded_shape=[128, 4, OK + OV], name=f"wst_{m}{cg}",
                )
                sync.dma_start(wst, src3[:, cg * 4 : (cg + 1) * 4, :])
                chunks.append(wst)
            return chunks

        def proj_tb(chunks, o_w, tb, ppool, evac):
            """x.T @ w for one token tile; evac(oc, psum_tile) per 512-chunk."""
            ncols = o_w // 512
            pps = [ppool.tile([128, 512], F32, tag="pp", name=f"pp{tb}_{i}") for i in range(ncols)]
            for hi in range(HI_N):
                xsl = xqT[:, hi, tb * 128 : (tb + 1) * 128]
                for oc in range(ncols):
                    nc.tensor.matmul(
                        pps[oc],
                        xsl,
                        chunks[hi // 4][:, hi % 4, oc * 512 : (oc + 1) * 512],
                        start=(hi == 0),
                        stop=(hi == HI_N - 1),
                    )
            for oc in range(ncols):
                evac(oc, pps[oc])

        with (
            tc.tile_pool(name="wmm1", bufs=5) as wmm1,
            tc.tile_pool(name="pproj", bufs=8, space="PSUM") as pproj,
            tc.tile_pool(name="ropep", bufs=1) as ropep,
        ):
            wst_kv = proj_load(wint_kv, OK + OV, "kv", wmm1)

            def evac_kv(tb):
                def f(oc, pp):
                    if oc < 2:  # k chunks
                        nc.vector.tensor_scalar(
                            k_sb[:, tb, oc * 512 : (oc + 1) * 512],
                            pp, dqv[("k", tb)], None, op0=OP.mult,
                        )
                    else:  # v chunks
                        nc.vector.tensor_scalar(
                            v_loc[:, tb, (oc - 2) * 4 : (oc - 1) * 4, :],
                            pp, dqv[("v", tb)], None, op0=OP.mult,
                        )
                return f

            for tb in range(TB):
                proj_tb(wst_kv, OK + OV, tb, pproj, evac_kv(tb))
                rope_batch(k_sb, tb, NKV, "ck", "sk", kT, ropep, "k")

            # bounce k/v to DRAM and allgather within the batch group
            nc.scalar.dma_start(
                kv_in[:, 0 : NKV * T].rearrange("p (hk t) -> p hk t", hk=NKV), kT
            )
            nc.scalar.dma_start(
                kv_in[:, NKV * T :].rearrange(
                    "p (a hk d) -> p a hk d", a=TB, hk=NKV
                ),
                v_loc,
            )
            nc.gpsimd.collective_compute(
                "AllGather", OP.bypass,
                replica_groups=[[0, 1, 2, 3], [4, 5, 6, 7]],
                ins=[kv_in.opt()], outs=[kv_out.opt()],
            )

            # ---- Q projection + rope (overlaps the KV allgather) ----
            wst_q = proj_load(wint_q, OQ, "q", wmm1)

            def evac_q(tb):
                def f(oc, pp):
                    nc.vector.tensor_scalar(
                        q_sb[:, tb, oc * 512 : (oc + 1) * 512],
                        pp, dqv[("q", tb)], None, op0=OP.mult,
                    )
                return f

            for tb in range(TB):
                proj_tb(wst_q, OQ, tb, pproj, evac_q(tb))
                rope_batch(q_sb, tb, NH, "cq", "sq", qT, ropep, "q")
            nc.gpsimd.collective_compute(
                "AllGather", OP.bypass, replica_groups=[list(range(NC))],
                ins=[wag_o.opt()], outs=[wint_o.opt()],
            )

        # ====== Phase A: attention ======
        with tc.tile_pool(name="wmm2", bufs=1) as wmm2:
            # prefetch o_proj weights under the attention phase
            src3o = wint_o.rearrange("(hi p) o -> p hi o", p=128)
            wsto = wmm2.tile([128, HI_N, OO], FP8, tag="wst2")
            sync.dma_start(wsto, src3o)

            # gather readback, keyed by (group-core z, slot a)
            KL = NKV * T  # k part length in kv_out cols
            for zc in range(GROUP):
                rows = kv_out[128 * zc : 128 * zc + 128, :]
                src_k = rows[:, 0:KL].rearrange("d (hk a t) -> d hk a t", hk=NKV, a=TB)
                nc.scalar.dma_start(kT_all[:, :, zc, :, :], src_k)
                src_v = rows[:, KL:].rearrange(
                    "p (a hk d) -> p a hk d", a=TB, hk=NKV
                )
                nc.scalar.dma_start(v_all[:, zc, :, :, 0:128], src_v)
            nc.vector.memset(v_all[:, :, :, :, 128:130], 1.0)
            # key tile j lives at (zj, aj)
            JZA = [(j, 0) if j < GROUP else (7 - j, 1) for j in range(KTILES)]

            with (
                tc.tile_pool(name="pscore", bufs=4, space="PSUM") as pscore,
                tc.tile_pool(name="ppv", bufs=3, space="PSUM") as ppv,
                tc.tile_pool(name="pexp", bufs=3) as pexp,
            ):
                pels = {}

                def scores(h):
                    hk = h // 2
                    pel = pexp.tile([128, KTILES, T], BF16, tag="pel")
                    pels[h] = pel
                    # slots 0..3: both q-tiles (N=256); slots 4..7: q-tile1 only
                    for g in range(2):
                        st = pscore.tile([128, 2, T], F32, tag="st")
                        for i in range(2):
                            zj, aj = JZA[2 * g + i]
                            nc.tensor.matmul(
                                st[:, i, :], kT_all[:, hk, zj, aj, :], qT[:, h, :],
                                start=True, stop=True,
                            )
                        nc.scalar.activation(
                            pel[:, 2 * g : 2 * g + 2, :], st, AF.Exp,
                            scale=INV_SQRT_HD,
                        )
                    for g in range(2, 4):
                        st = pscore.tile([128, 2, T], F32, tag="st")
                        for i in range(2):
                            zj, aj = JZA[2 * g + i]
                            nc.tensor.matmul(
                                st[:, i, 128:256],
                                kT_all[:, hk, zj, aj, :], qT[:, h, 128:256],
                                start=True, stop=True,
                            )
                        nc.scalar.activation(
                            pel[:, 2 * g : 2 * g + 2, 128:256],
                            st[:, :, 128:256], AF.Exp, scale=INV_SQRT_HD,
                        )
                    # per-core causal masks: q-tile0 all 4 slots; q-tile1 slots 4-7
                    nc.vector.tensor_mul(
                        pel[:, 0:NJ0, 0:128], pel[:, 0:NJ0, 0:128], mask_sb[:, 0, :, :]
                    )
                    nc.vector.tensor_mul(
                        pel[:, NJ0:KTILES, 128:256],
                        pel[:, NJ0:KTILES, 128:256],
                        mask_sb[:, 1, :, :],
                    )

                def pv(h):
                    hk = h // 2
                    pel = pels.pop(h)
                    for a, nj in ((0, NJ0), (1, NJ1)):
                        po = ppv.tile([128, 132], F32, tag="po")
                        for j in range(nj):
                            zj, aj = JZA[j]
                            nc.tensor.matmul(
                                po[:, 0:129],
                                pel[:, j, a * 128 : (a + 1) * 128],
                                v_all[:, zj, aj, hk, 0:129],
                                start=(j == 0),
                                stop=(j == nj - 1),
                            )
                        rden = vecs.tile([128, 1], F32, name=f"rden{h}_{a}")
                        nc.vector.reciprocal(rden, po[:, 128:129])
                        nc.vector.tensor_scalar(
                            attn[:, a, h * 128 : (h + 1) * 128],
                            po[:, 0:128], rden, None, op0=OP.mult,
                        )

                # software-pipelined: scores one head ahead of PV
                scores(0)
                for h in range(NH):
                    if h + 1 < NH:
                        scores(h + 1)
                    pv(h)

            # ====== Phase O: act_quant(attn) + o_proj ======
            with (
                tc.tile_pool(name="oq", bufs=2) as oq,
                tc.tile_pool(name="pproj2", bufs=4, space="PSUM") as pproj2,
                tc.tile_pool(name="osb", bufs=2) as osb,
            ):
                dqo = []
                for tb in range(TB):
                    axm = vecs.tile([128, 1], F32, name=f"oaxm{tb}")
                    nc.vector.tensor_reduce(
                        axm, attn[:, tb, :], axis=AX, op=OP.max,
                        apply_absolute_value=True,
                    )
                    rsx = vecs.tile([128, 1], F32, name=f"orsx{tb}")
                    nc.vector.reciprocal(rsx, axm)
                    sxq = vecs.tile([128, 1], F32, name=f"osxq{tb}")
                    nc.vector.tensor_scalar_mul(sxq, rsx, 127.0)
                    dq = vecs.tile([128, 1], F32, name=f"odqx{tb}")
                    nc.vector.tensor_scalar_mul(dq, axm, 1.0 / 127.0)
                    d2 = vecs.tile([128, 1], F32, name=f"odq2{tb}")
                    nc.vector.tensor_mul(d2, dq, rswb["o"])
                    dqo.append(d2)
                    ar = oq.tile([128, H], F32, tag="ar")
                    nc.vector.tensor_scalar(
                        ar, attn[:, tb, :], sxq, RND, op0=OP.mult, op1=OP.add
                    )
                    aqb = oq.tile([128, H], BF16, tag="aqb")
                    nc.scalar.activation(aqb, ar, AF.Identity, bias=negrnd)
                    sync.dma_start(
                        aT[:, :, tb * 128 : (tb + 1) * 128], aqb, transpose=True
                    )

                for tb in range(TB):
                    pps = [pproj2.tile([128, 512], F32, tag="pp2", name=f"pp2_{tb}_{i}") for i in range(4)]
                    for hi in range(HI_N):
                        asl = aT[:, hi, tb * 128 : (tb + 1) * 128]
                        for oc in range(4):
                            nc.tensor.matmul(
                                pps[oc],
                                asl,
                                wsto[:, hi, oc * 512 : (oc + 1) * 512],
                                start=(hi == 0),
                                stop=(hi == HI_N - 1),
                            )
                    for oc in range(4):
                        ot = osb.tile([128, 512], F32, tag="ot")
                        nc.vector.tensor_scalar(ot, pps[oc], dqo[tb], None, op0=OP.mult)
                        sync.dma_start(
                            out.ap()[
                                tb * 128 : (tb + 1) * 128, oc * 512 : (oc + 1) * 512
                            ],
                            ot,
                        )


def _host_inputs(x, cos, sin, wq, wk, wv, wo, qn, kn):
    """Build the 8 per-core input maps (pure slicing / layout transforms)."""
    x2 = np.asarray(x, np.float32).reshape(B * S, H)
    cos = np.asarray(cos, np.float32)
    sin = np.asarray(sin, np.float32)
    qn = np.asarray(qn, np.float32)
    kn = np.asarray(kn, np.float32)
    # fold qk-norm weights into rope tables (exact identity when qn=kn=1)
    qn_rot = np.concatenate([qn[HD // 2 :], qn[: HD // 2]])
    kn_rot = np.concatenate([kn[HD // 2 :], kn[: HD // 2]])
    sgn = np.concatenate(
        [-np.ones(HD // 2, np.float32), np.ones(HD // 2, np.float32)]
    )
    cosq_t = cos * qn[None, :]
    sinq_t = sin * (qn_rot * sgn)[None, :]
    cosk_t = cos * kn[None, :]
    sink_t = sin * (kn_rot * sgn)[None, :]

    wt = {
        "q": np.asarray(wq, np.float32).T,  # [H, OQ]
        "k": np.asarray(wk, np.float32).T,
        "v": np.asarray(wv, np.float32).T,
        "o": np.asarray(wo, np.float32).T,  # [H(=in), OO]
    }
    worder = ("k", "v", "q", "o")
    wconst = np.concatenate(
        [
            np.array([WNUMEL[m] for m in worder], np.float32),
            np.array([1.0 / WNUMEL[m] for m in worder], np.float32),
        ]
    ).reshape(1, 8)

    p = np.arange(128)[:, None]
    f = np.arange(128)[None, :]
    tri = (p <= f)  # pel[k, q] upper-incl triangle within the diagonal tile

    in_maps = []
    for c in range(NC):
        b, z = c // GROUP, c % GROUP
        t0a = b * S + z * 128  # q-tile0 = batch tile z
        t0b = b * S + (7 - z) * 128  # q-tile1 = batch tile 7-z
        rows = np.r_[t0a : t0a + 128, t0b : t0b + 128]
        # masks: [128 k, 2, 4, 128 q]
        mask = np.zeros((128, 2, NJ0, 128), np.float32)
        for j in range(NJ0):  # q-tile0 (tile z) vs key tiles 0..3
            if j < z:
                mask[:, 0, j, :] = 1.0
            elif j == z:
                mask[:, 0, j, :] = tri
        for j in range(NJ0, KTILES):  # q-tile1 (tile 7-z) vs key tiles 4..7
            if j < 7 - z:
                mask[:, 1, j - NJ0, :] = 1.0
            elif j == 7 - z:
                mask[:, 1, j - NJ0, :] = tri
        pos = np.r_[z * 128 : z * 128 + 128, (7 - z) * 128 : (8 - z) * 128]
        m = {
            "x_sl": np.ascontiguousarray(x2[rows]),
            "cosq": np.ascontiguousarray(cosq_t[pos]),
            "sinq": np.ascontiguousarray(sinq_t[pos]),
            "cosk": np.ascontiguousarray(cosk_t[pos]),
            "sink": np.ascontiguousarray(sink_t[pos]),
            "wq_sl": np.ascontiguousarray(wt["q"][c * HSL : (c + 1) * HSL]),
            "wk_sl": np.ascontiguousarray(wt["k"][c * HSL : (c + 1) * HSL]),
            "wv_sl": np.ascontiguousarray(wt["v"][c * HSL : (c + 1) * HSL]),
            "wo_sl": np.ascontiguousarray(wt["o"][c * HSL : (c + 1) * HSL]),
            "mask": mask.astype(ml_dtypes.bfloat16),
            "wconst": wconst,
        }
        in_maps.append(m)
    return in_maps


def kernel(x, cos, sin, wq, wk, wv, wo, qn, kn):
    if "nc" not in _CACHE:
        _CACHE["nc"] = _build()
    nc = _CACHE["nc"]
    in_maps = _host_inputs(x, cos, sin, wq, wk, wv, wo, qn, kn)
    res = bass_utils.run_bass_kernel_spmd(nc, in_maps, core_ids=list(range(NC)))
    full = np.zeros((B * S, H), np.float32)
    for c in range(NC):
        b, z = c // GROUP, c % GROUP
        o = np.asarray(res.results[c]["out"])
        t0a = b * S + z * 128
        t0b = b * S + (7 - z) * 128
        full[t0a : t0a + 128] = o[0:128]
        full[t0b : t0b + 128] = o[128:256]
    return full.reshape(B, S, H)


# revision 16
# speedup vs baseline: 1.0399x; 1.0399x over previous
"""BitNet GQA attention layer on 8 TRN2 NeuronCores — v3.

Sharding: token-parallel with zigzag causal balance. B*S = 2048 tokens ->
256 per core. Core c (batch b=c//4, zig z=c%4) owns query tiles {z, 7-z}
of batch b, so every core computes the same 12 score blocks per head
(4 for tile z masked per-core, 8 for tile 7-z) instead of the naive 20.

Weights are split 8-way along the contraction dim for quantization (one
tiny AllReduce for all four matrices' abs-sums, preceded by a dummy
collective that absorbs the first-op setup cost), then ternary fp8
weights are AllGathered (k+v merged, then q, then o last). K/V
activations are AllGathered within each batch's 4-core group (k and v
merged into one collective). Transposes run on the PE (fp32 for the
integer activations with the round-bias fold, bf16 for rope outputs).
BitNet matmuls are exact integer arithmetic in bf16 x fp8 with fp32
PSUM accumulation; projection loops run hi-outer/chunk-inner and the
stationary activation slices are contiguous 128-col blocks (FWL).
"""

import sys

sys.path.insert(0, "/opt/trn_rl_repo")

import numpy as np
import ml_dtypes

import concourse.bass as bass
import concourse.mybir as mybir
import concourse.tile as tile
from concourse import bacc
from concourse import bass_utils
from concourse.masks import make_identity

F32 = mybir.dt.float32
BF16 = mybir.dt.bfloat16
FP8 = mybir.dt.float8e4
AX = mybir.AxisListType.X
OP = mybir.AluOpType
AF = mybir.ActivationFunctionType

B, S, H = 2, 1024, 2048
NH, NKV, HD = 16, 8, 128
NC = 8
T = (B * S) // NC  # 256 tokens per core
TB = T // 128  # 2 token tiles per core
HSL = H // NC  # 256 weight rows per core
EPS = 1e-6
RND = 12582912.0  # 1.5 * 2**23: fp32 add => round-to-nearest-even
INV_SQRT_HD = 1.0 / float(np.sqrt(HD))
KTILES = S // 128  # 8 key tiles per batch
GROUP = 4  # cores per batch
NJ0 = 4  # key slots computed for q-tile0 (covers z <= 3)
NJ1 = KTILES  # key slots computed for q-tile1

OQ, OK, OV, OO = H, NKV * HD, NKV * HD, H  # 2048, 1024, 1024, 2048
OW = {"q": OQ, "k": OK, "v": OV, "o": OO}
WNUMEL = {m: OW[m] * H for m in OW}
HI_N = H // 128  # 16 contraction tiles

_CACHE = {}


def _build():
    nc = bacc.Bacc("TRN2", target_bir_lowering=False, debug=False, num_devices=NC)

    x_sl = nc.dram_tensor("x_sl", [T, H], F32, kind="ExternalInput")
    cosq = nc.dram_tensor("cosq", [T, HD], F32, kind="ExternalInput")
    sinq = nc.dram_tensor("sinq", [T, HD], F32, kind="ExternalInput")
    cosk = nc.dram_tensor("cosk", [T, HD], F32, kind="ExternalInput")
    sink = nc.dram_tensor("sink", [T, HD], F32, kind="ExternalInput")
    w_sl = {
        "q": nc.dram_tensor("wq_sl", [HSL, OQ], F32, kind="ExternalInput"),
        "k": nc.dram_tensor("wk_sl", [HSL, OK], F32, kind="ExternalInput"),
        "v": nc.dram_tensor("wv_sl", [HSL, OV], F32, kind="ExternalInput"),
        "o": nc.dram_tensor("wo_sl", [HSL, OO], F32, kind="ExternalInput"),
    }
    # mask[p, a, j, f]: a=0 -> pel[k=p, slot j (abs key tile j), q=f] of tile z
    #                  a=1 -> slots 4..7 (abs key tiles 4..7) of tile 7-z
    mask_in = nc.dram_tensor("mask", [128, 2, NJ0, 128], BF16, kind="ExternalInput")
    # cols 0-3: numel for k,v,q,o ; cols 4-7: 1/numel for k,v,q,o
    wconst = nc.dram_tensor("wconst", [1, 8], F32, kind="ExternalInput")
    out = nc.dram_tensor("out", [T, H], F32, kind="ExternalOutput")

    with tile.TileContext(nc) as tc:
        _build_body(nc, tc, x_sl, cosq, sinq, cosk, sink, w_sl, mask_in, wconst, out)

    nc.compile()
    return nc


def _build_body(nc, tc, x_sl, cosq, sinq, cosk, sink, w_sl, mask_in, wconst, out):
    sync = nc.sync

    with (
        tc.tile_pool(name="dram", bufs=1, space="DRAM") as dram,
        tc.tile_pool(name="const", bufs=1) as constp,
        tc.tile_pool(name="vecs", bufs=1) as vecs,
        tc.tile_pool(name="persist", bufs=1) as persist,
        tc.tile_pool(name="ptrans", bufs=2, space="PSUM") as ptrans,
    ):
        # ---- DRAM bounce buffers for collectives ----
        dum_in = dram.tile([1, 8], F32)
        dum_out = dram.tile([8, 8], F32, addr_space="Shared")
        wag_kv = dram.tile([HSL, OK + OV], FP8)
        wint_kv = dram.tile([H, OK + OV], FP8, addr_space="Shared")
        wag_q = dram.tile([HSL, OQ], FP8)
        wint_q = dram.tile([H, OQ], FP8, addr_space="Shared")
        wag_o = dram.tile([HSL, OO], FP8)
        wint_o = dram.tile([H, OO], FP8, addr_space="Shared")
        ar_in = dram.tile([1, 8], F32)
        ar_out = dram.tile([1, 8], F32, addr_space="Shared")
        # cols 0:2048 = kT (hk,t); cols 2048:4096 = v (a,hk,d)
        kv_in = dram.tile([128, NKV * T + TB * NKV * HD], BF16)
        kv_out = dram.tile([512, NKV * T + TB * NKV * HD], BF16)

        # warm up the collective path while the weights stream in
        nc.gpsimd.collective_compute(
            "AllGather", OP.bypass, replica_groups=[list(range(NC))],
            ins=[dum_in.opt()], outs=[dum_out.opt()],
        )

        # ---- constants ----
        ones1 = constp.tile([1, 128], F32)
        nc.vector.memset(ones1, 1.0)
        onescol = constp.tile([128, 1], F32)
        nc.vector.memset(onescol, 1.0)
        wconst_sb = constp.tile([1, 8], F32)
        sync.dma_start(wconst_sb, wconst.ap())
        negrnd = constp.tile([128, 1], F32)
        nc.vector.memset(negrnd, -RND)
        epsb = constp.tile([128, 1], F32)
        nc.vector.memset(epsb, EPS)
        ident = constp.tile([128, 128], F32)
        make_identity(nc, ident)
        identb = constp.tile([128, 128], BF16)
        make_identity(nc, identb)
        mask_sb = constp.tile([128, 2, NJ0, 128], BF16)
        sync.dma_start(mask_sb, mask_in.ap())
        cs = {}
        for nm, t in (("cq", cosq), ("sq", sinq), ("ck", cosk), ("sk", sink)):
            c = constp.tile([128, TB, HD], F32, name=f"cs_{nm}")
            sync.dma_start(c, t.ap().rearrange("(a p) d -> p a d", p=128))
            cs[nm] = c

        # persistent activations (stationary slices contiguous: [.., tb, hi, 128])
        xqT = persist.tile([128, TB, HI_N, 128], BF16)
        qT = persist.tile([128, NH, T], BF16)  # [d, head, t]
        kT = persist.tile([128, NKV, T], BF16)  # [d, kv head, t] (local)
        v_loc = persist.tile([128, TB, NKV, HD], BF16)
        q_sb = persist.tile([128, TB, OQ], F32)
        k_sb = persist.tile([128, TB, OK], F32)
        attn = persist.tile([128, TB, H], F32)
        aT = persist.tile([128, TB, HI_N, 128], BF16)
        # gathered k/v keyed by (group-core z, slot a); key tile j = (zj, aj)
        kT_all = persist.tile([128, NKV, GROUP, TB, 128], BF16)
        v_all = persist.tile([128, GROUP, TB, NKV, 130], BF16)

        # ====== Phase W: x quant/transpose + weight scales + quant + AGs ======
        dqx = []
        with (
            tc.tile_pool(name="xraw", bufs=2) as xraw,
            tc.tile_pool(name="wraw_k", bufs=1) as wraw_k,
            tc.tile_pool(name="wraw_v", bufs=1) as wraw_v,
            tc.tile_pool(name="wraw_q", bufs=1) as wraw_q,
            tc.tile_pool(name="wraw_o", bufs=1) as wraw_o,
            tc.tile_pool(name="scr", bufs=2) as scr,
            tc.tile_pool(name="wtmp", bufs=1) as wtmp,
            tc.tile_pool(name="wq8", bufs=2) as wq8,
            tc.tile_pool(name="psmall", bufs=2, space="PSUM") as psmall,
        ):
            # ---- x: load + act_quant + PE transpose ----
            xs_t = []
            for tb in range(TB):
                xs = xraw.tile([128, H], F32, tag="xs", name=f"xs{tb}")
                sync.dma_start(xs, x_sl.ap()[tb * 128 : (tb + 1) * 128, :])
                xs_t.append(xs)

            wraws = {"k": wraw_k, "v": wraw_v, "q": wraw_q, "o": wraw_o}
            worder = ("k", "v", "q", "o")
            wab = {}
            for m in worder:
                for pt in range(2):
                    wr = wraws[m].tile([128, OW[m]], F32, name=f"wr_{m}{pt}")
                    sync.dma_start(wr, w_sl[m].ap()[pt * 128 : (pt + 1) * 128, :])
                    wab[(m, pt)] = wr

            for tb in range(TB):
                xs = xs_t[tb]
                axm = vecs.tile([128, 1], F32, name=f"axm{tb}")
                nc.vector.tensor_reduce(
                    axm, xs, axis=AX, op=OP.max, apply_absolute_value=True
                )
                rsx = vecs.tile([128, 1], F32, name=f"rsx{tb}")
                nc.vector.reciprocal(rsx, axm)
                sxq = vecs.tile([128, 1], F32, name=f"sxq{tb}")
                nc.vector.tensor_scalar_mul(sxq, rsx, 127.0)
                dq = vecs.tile([128, 1], F32, name=f"dqx{tb}")
                nc.vector.tensor_scalar_mul(dq, axm, 1.0 / 127.0)
                dqx.append(dq)
                nc.vector.tensor_scalar(
                    xs, xs, sxq, RND, op0=OP.mult, op1=OP.add
                )
                for hg in range(0, HI_N, 4):
                    pt4 = ptrans.tile([128, 4, 128], F32, tag="ptr")
                    for i in range(4):
                        hi = hg + i
                        nc.tensor.transpose(
                            pt4[:, i, :], xs[:, hi * 128 : (hi + 1) * 128], ident
                        )
                    nc.scalar.activation(
                        xqT[:, tb, hg : hg + 4, :], pt4, AF.Identity, bias=negrnd
                    )

            # ---- weight abs-sums on the scalar engine (accumulate output) ----
            red0 = vecs.tile([128, 4], F32, name="red0")
            red1 = vecs.tile([128, 4], F32, name="red1")
            for mi, m in enumerate(worder):
                for pt, red in ((0, red0), (1, red1)):
                    sc = scr.tile([128, OW[m]], F32, tag="scr", name=f"sc_{m}{pt}")
                    nc.scalar.activation(
                        sc, wab[(m, pt)], AF.Abs, accum_out=red[:, mi : mi + 1]
                    )
            redc = vecs.tile([128, 4], F32, name="redc")
            nc.vector.tensor_add(redc, red0, red1)
            ps = psmall.tile([1, 4], F32, name="ps_sums", tag="psm")
            nc.tensor.matmul(ps, onescol, redc, start=True, stop=True)
            sums = vecs.tile([1, 8], F32, name="sums")
            nc.vector.memset(sums, 0.0)
            nc.scalar.copy(sums[:, 0:4], ps)
            nc.scalar.dma_start(ar_in, sums)
            nc.gpsimd.collective_compute(
                "AllReduce", OP.add, replica_groups=[list(range(NC))],
                ins=[ar_in.opt()], outs=[ar_out.opt()],
            )

            # ---- scales from the AllReduce ----
            g = vecs.tile([1, 8], F32, name="g")
            nc.scalar.dma_start(g, ar_out)
            r4 = vecs.tile([1, 4], F32, name="r4")
            nc.vector.reciprocal(r4, g[:, 0:4])
            sw8 = vecs.tile([1, 8], F32, name="sw8")
            nc.vector.tensor_mul(sw8[:, 0:4], r4, wconst_sb[:, 0:4])
            nc.vector.tensor_mul(sw8[:, 4:8], g[:, 0:4], wconst_sb[:, 4:8])
            pb = psmall.tile([128, 8], F32, name="pb", tag="psm")
            nc.tensor.matmul(pb, ones1, sw8, start=True, stop=True)
            sb8 = vecs.tile([128, 8], F32, name="sb8")
            nc.scalar.copy(sb8, pb)
            rswb = {m: sb8[:, 4 + mi : 5 + mi] for mi, m in enumerate(worder)}

            def w_quant(m, mi, dst, col0):
                for pt in range(2):
                    wr = wab[(m, pt)]
                    t1 = wtmp.tile([128, OW[m]], F32, tag="wtmp", name=f"t1_{m}{pt}")
                    nc.vector.tensor_scalar(
                        t1, wr, sb8[:, mi : mi + 1], RND, op0=OP.mult, op1=OP.add
                    )
                    nc.scalar.activation(t1, t1, AF.Identity, bias=negrnd)
                    wi = wq8.tile([128, OW[m]], FP8, tag="wi")
                    nc.vector.tensor_scalar(wi, t1, 1.0, -1.0, op0=OP.min, op1=OP.max)
                    nc.scalar.dma_start(
                        dst[pt * 128 : (pt + 1) * 128, col0 : col0 + OW[m]], wi
                    )

            w_quant("k", 0, wag_kv, 0)
            w_quant("v", 1, wag_kv, OK)
            nc.gpsimd.collective_compute(
                "AllGather", OP.bypass, replica_groups=[list(range(NC))],
                ins=[wag_kv.opt()], outs=[wint_kv.opt()],
            )
            w_quant("q", 2, wag_q, 0)
            nc.gpsimd.collective_compute(
                "AllGather", OP.bypass, replica_groups=[list(range(NC))],
                ins=[wag_q.opt()], outs=[wint_q.opt()],
            )
            w_quant("o", 3, wag_o, 0)

        # dequant vectors (absmax/127 * 1/s_w)
        dqv = {}
        for m in ("q", "k", "v", "o"):
            for tb in range(TB):
                d = vecs.tile([128, 1], F32, name=f"dqv_{m}{tb}")
                nc.vector.tensor_mul(d, dqx[tb], rswb[m])
                dqv[(m, tb)] = d

        def rope_batch(src_sb, tb, nh, cosn, sinn, dstT, ropep, label):
            w = nh * 128
            blk = src_sb[:, tb, :]  # [128, w] f32
            sq = ropep.tile([128, w], F32, tag="unf", padded_shape=[128, NH * 128])
            nc.scalar.activation(sq, blk, AF.Square)
            ms = vecs.tile([128, nh], F32, name=f"ms_{label}{tb}")
            nc.vector.tensor_reduce(
                ms, sq.rearrange("p (h d) -> p h d", h=nh), axis=AX, op=OP.add
            )
            rms = vecs.tile([128, nh], F32, name=f"rms_{label}{tb}")
            nc.scalar.activation(rms, ms, AF.Sqrt, scale=1.0 / HD, bias=epsb)
            rn = vecs.tile([128, nh], F32, name=f"rn_{label}{tb}")
            nc.vector.reciprocal(rn, rms)
            rnb = rn.to_broadcast([128, nh, 128])
            blk3 = blk.rearrange("p (h d) -> p h d", h=nh)
            un2 = ropep.tile(
                [128, nh * 128], F32, tag="unf", padded_shape=[128, NH * 128],
                name="un2",
            )
            un = un2.rearrange("p (h d) -> p h d", h=nh)
            nc.vector.tensor_mul(un, blk3, rnb)
            cosb = (
                cs[cosn][:, tb, :]
                .rearrange("p (one d) -> p one d", one=1)
                .to_broadcast([128, nh, 128])
            )
            sinb = (
                cs[sinn][:, tb, :]
                .rearrange("p (one d) -> p one d", one=1)
                .to_broadcast([128, nh, 128])
            )
            ra2 = ropep.tile([128, nh * 128], F32, tag="ra", padded_shape=[128, NH * 128])
            ra = ra2.rearrange("p (h d) -> p h d", h=nh)
            nc.vector.tensor_mul(ra, un, cosb)
            rb2 = ropep.tile([128, nh * 128], F32, tag="rb", padded_shape=[128, NH * 128])
            rb = rb2.rearrange("p (h d) -> p h d", h=nh)
            nc.vector.tensor_mul(rb[:, :, 0:64], un[:, :, 64:128], sinb[:, :, 0:64])
            nc.vector.tensor_mul(rb[:, :, 64:128], un[:, :, 0:64], sinb[:, :, 64:128])
            raf = ropep.tile(
                [128, nh * 128], BF16, tag="raf", padded_shape=[128, NH * 128]
            )
            nc.vector.tensor_add(raf, ra2, rb2)
            for hg in range(0, nh, 4):
                pt4 = ptrans.tile(
                    [128, 4, 128], BF16, tag="ptr", padded_shape=[128, 4, 256]
                )
                for i in range(4):
                    nc.tensor.transpose(
                        pt4[:, i, :], raf[:, (hg + i) * 128 : (hg + i + 1) * 128],
                        identb,
                    )
                nc.scalar.activation(
                    dstT[:, hg : hg + 4, tb * 128 : (tb + 1) * 128], pt4, AF.Copy
                )

        # ====== Phase KV/Q: projections (hi-outer, chunk-inner) ======
        def proj_load(wint_src, o_w, m, wpool):
            src3 = wint_src.rearrange("(hi p) o -> p hi o", p=128)
            chunks = []
            for cg in range(4):
                wst = wpool.tile(
                    [128, 4, o_w], FP8, tag="wst",
                    padded_shape=[128, 4, OK + OV], name=f"wst_{m}{cg}",
                )
                sync.dma_start(wst, src3[:, cg * 4 : (cg + 1) * 4, :])
                chunks.append(wst)
            return chunks

        def proj_tb(chunks, o_w, tb, ppool, evac):
            """x.T @ w for one token tile; evac(oc, psum_tile) per 512-chunk."""
            ncols = o_w // 512
            pps = [
                ppool.tile([128, 512], F32, tag="pp", name=f"pp{tb}_{i}")
                for i in range(ncols)
            ]
            for hi in range(HI_N):
                xsl = xqT[:, tb, hi, :]
                for oc in range(ncols):
                    nc.tensor.matmul(
                        pps[oc],
                        xsl,
                        chunks[hi // 4][:, hi % 4, oc * 512 : (oc + 1) * 512],
                        start=(hi == 0),
                        stop=(hi == HI_N - 1),
                    )
            for oc in range(ncols):
                evac(oc, pps[oc])

        with (
            tc.tile_pool(name="wmm1", bufs=5) as wmm1,
            tc.tile_pool(name="pproj", bufs=6, space="PSUM") as pproj,
            tc.tile_pool(name="ropep", bufs=1) as ropep,
        ):
            wst_kv = proj_load(wint_kv, OK + OV, "kv", wmm1)

            def evac_kv(tb):
                def f(oc, pp):
                    if oc < 2:  # k chunks
                        nc.vector.tensor_scalar(
                            k_sb[:, tb, oc * 512 : (oc + 1) * 512],
                            pp, dqv[("k", tb)], None, op0=OP.mult,
                        )
                    else:  # v chunks
                        nc.vector.tensor_scalar(
                            v_loc[:, tb, (oc - 2) * 4 : (oc - 1) * 4, :],
                            pp, dqv[("v", tb)], None, op0=OP.mult,
                        )
                return f

            for tb in range(TB):
                proj_tb(wst_kv, OK + OV, tb, pproj, evac_kv(tb))
                rope_batch(k_sb, tb, NKV, "ck", "sk", kT, ropep, "k")

            # bounce k/v to DRAM and allgather within the batch group
            nc.scalar.dma_start(
                kv_in[:, 0 : NKV * T].rearrange("p (hk t) -> p hk t", hk=NKV), kT
            )
            nc.scalar.dma_start(
                kv_in[:, NKV * T :].rearrange(
                    "p (a hk d) -> p a hk d", a=TB, hk=NKV
                ),
                v_loc,
            )
            nc.gpsimd.collective_compute(
                "AllGather", OP.bypass,
                replica_groups=[[0, 1, 2, 3], [4, 5, 6, 7]],
                ins=[kv_in.opt()], outs=[kv_out.opt()],
            )

            # ---- Q projection + rope (overlaps the KV allgather) ----
            wst_q = proj_load(wint_q, OQ, "q", wmm1)

            def evac_q(tb):
                def f(oc, pp):
                    nc.vector.tensor_scalar(
                        q_sb[:, tb, oc * 512 : (oc + 1) * 512],
                        pp, dqv[("q", tb)], None, op0=OP.mult,
                    )
                return f

            for tb in range(TB):
                proj_tb(wst_q, OQ, tb, pproj, evac_q(tb))
                rope_batch(q_sb, tb, NH, "cq", "sq", qT, ropep, "q")
            nc.gpsimd.collective_compute(
                "AllGather", OP.bypass, replica_groups=[list(range(NC))],
                ins=[wag_o.opt()], outs=[wint_o.opt()],
            )

        # ====== Phase A: attention ======
        with tc.tile_pool(name="wmm2", bufs=1) as wmm2:
            # prefetch o_proj weights under the attention phase
            src3o = wint_o.rearrange("(hi p) o -> p hi o", p=128)
            wsto = wmm2.tile([128, HI_N, OO], FP8, tag="wst2")
            sync.dma_start(wsto, src3o)

            # gather readback, keyed by (group-core z, slot a)
            KL = NKV * T  # k part length in kv_out cols
            for zc in range(GROUP):
                rows = kv_out[128 * zc : 128 * zc + 128, :]
                src_k = rows[:, 0:KL].rearrange("d (hk a t) -> d hk a t", hk=NKV, a=TB)
                nc.scalar.dma_start(kT_all[:, :, zc, :, :], src_k)
                src_v = rows[:, KL:].rearrange(
                    "p (a hk d) -> p a hk d", a=TB, hk=NKV
                )
                nc.scalar.dma_start(v_all[:, zc, :, :, 0:128], src_v)
            nc.vector.memset(v_all[:, :, :, :, 128:130], 1.0)
            # key tile j lives at (zj, aj)
            JZA = [(j, 0) if j < GROUP else (7 - j, 1) for j in range(KTILES)]

            with (
                tc.tile_pool(name="pscore", bufs=3, space="PSUM") as pscore,
                tc.tile_pool(name="ppv", bufs=3, space="PSUM") as ppv,
                tc.tile_pool(name="pexp", bufs=3) as pexp,
            ):
                pels = {}

                def scores(h):
                    hk = h // 2
                    pel = pexp.tile([128, KTILES, T], BF16, tag="pel")
                    pels[h] = pel
                    # slots 0..3: both q-tiles (N=256); slots 4..7: q-tile1 only
                    for g in range(2):
                        st = pscore.tile([128, 2, T], F32, tag="st")
                        for i in range(2):
                            zj, aj = JZA[2 * g + i]
                            nc.tensor.matmul(
                                st[:, i, :], kT_all[:, hk, zj, aj, :], qT[:, h, :],
                                start=True, stop=True,
                            )
                        nc.scalar.activation(
                            pel[:, 2 * g : 2 * g + 2, :], st, AF.Exp,
                            scale=INV_SQRT_HD,
                        )
                    for g in range(2, 4):
                        st = pscore.tile([128, 2, T], F32, tag="st")
                        for i in range(2):
                            zj, aj = JZA[2 * g + i]
                            nc.tensor.matmul(
                                st[:, i, 128:256],
                                kT_all[:, hk, zj, aj, :], qT[:, h, 128:256],
                                start=True, stop=True,
                            )
                        nc.scalar.activation(
                            pel[:, 2 * g : 2 * g + 2, 128:256],
                            st[:, :, 128:256], AF.Exp, scale=INV_SQRT_HD,
                        )
                    # per-core causal masks: q-tile0 all 4 slots; q-tile1 slots 4-7
                    nc.vector.tensor_mul(
                        pel[:, 0:NJ0, 0:128], pel[:, 0:NJ0, 0:128], mask_sb[:, 0, :, :]
                    )
                    nc.vector.tensor_mul(
                        pel[:, NJ0:KTILES, 128:256],
                        pel[:, NJ0:KTILES, 128:256],
                        mask_sb[:, 1, :, :],
                    )

                def pv(h):
                    hk = h // 2
                    pel = pels.pop(h)
                    for a, nj in ((0, NJ0), (1, NJ1)):
                        po = ppv.tile([128, 132], F32, tag="po")
                        for j in range(nj):
                            zj, aj = JZA[j]
                            nc.tensor.matmul(
                                po[:, 0:129],
                                pel[:, j, a * 128 : (a + 1) * 128],
                                v_all[:, zj, aj, hk, 0:129],
                                start=(j == 0),
                                stop=(j == nj - 1),
                            )
                        rden = vecs.tile([128, 1], F32, name=f"rden{h}_{a}")
                        nc.vector.reciprocal(rden, po[:, 128:129])
                        nc.vector.tensor_scalar(
                            attn[:, a, h * 128 : (h + 1) * 128],
                            po[:, 0:128], rden, None, op0=OP.mult,
                        )

                # software-pipelined: scores one head ahead of PV
                scores(0)
                for h in range(NH):
                    if h + 1 < NH:
                        scores(h + 1)
                    pv(h)

            # ====== Phase O: act_quant(attn) + o_proj ======
            with (
                tc.tile_pool(name="oq", bufs=2) as oq,
                tc.tile_pool(name="pproj2", bufs=4, space="PSUM") as pproj2,
                tc.tile_pool(name="osb", bufs=2) as osb,
            ):
                dqo = []
                for tb in range(TB):
                    axm = vecs.tile([128, 1], F32, name=f"oaxm{tb}")
                    nc.vector.tensor_reduce(
                        axm, attn[:, tb, :], axis=AX, op=OP.max,
                        apply_absolute_value=True,
                    )
                    rsx = vecs.tile([128, 1], F32, name=f"orsx{tb}")
                    nc.vector.reciprocal(rsx, axm)
                    sxq = vecs.tile([128, 1], F32, name=f"osxq{tb}")
                    nc.vector.tensor_scalar_mul(sxq, rsx, 127.0)
                    dq = vecs.tile([128, 1], F32, name=f"odqx{tb}")
                    nc.vector.tensor_scalar_mul(dq, axm, 1.0 / 127.0)
                    d2 = vecs.tile([128, 1], F32, name=f"odq2{tb}")
                    nc.vector.tensor_mul(d2, dq, rswb["o"])
                    dqo.append(d2)
                    ar = oq.tile([128, H], F32, tag="ar")
                    nc.vector.tensor_scalar(
                        ar, attn[:, tb, :], sxq, RND, op0=OP.mult, op1=OP.add
                    )
                    for hg in range(0, HI_N, 4):
                        pt4 = ptrans.tile([128, 4, 128], F32, tag="ptr")
                        for i in range(4):
                            hi = hg + i
                            nc.tensor.transpose(
                                pt4[:, i, :], ar[:, hi * 128 : (hi + 1) * 128], ident
                            )
                        nc.scalar.activation(
                            aT[:, tb, hg : hg + 4, :], pt4, AF.Identity, bias=negrnd
                        )

                for tb in range(TB):
                    pps = [
                        pproj2.tile([128, 512], F32, tag="pp2", name=f"pp2_{tb}_{i}")
                        for i in range(4)
                    ]
                    for hi in range(HI_N):
                        asl = aT[:, tb, hi, :]
                        for oc in range(4):
                            nc.tensor.matmul(
                                pps[oc],
                                asl,
                                wsto[:, hi, oc * 512 : (oc + 1) * 512],
                                start=(hi == 0),
                                stop=(hi == HI_N - 1),
                            )
                    for oc in range(4):
                        ot = osb.tile([128, 512], F32, tag="ot")
                        nc.vector.tensor_scalar(ot, pps[oc], dqo[tb], None, op0=OP.mult)
                        sync.dma_start(
                            out.ap()[
                                tb * 128 : (tb + 1) * 128, oc * 512 : (oc + 1) * 512
                            ],
                            ot,
                        )


def _host_inputs(x, cos, sin, wq, wk, wv, wo, qn, kn):
    """Build the 8 per-core input maps (pure slicing / layout transforms)."""
    x2 = np.asarray(x, np.float32).reshape(B * S, H)
    cos = np.asarray(cos, np.float32)
    sin = np.asarray(sin, np.float32)
    qn = np.asarray(qn, np.float32)
    kn = np.asarray(kn, np.float32)
    # fold qk-norm weights into rope tables (exact identity when qn=kn=1)
    qn_rot = np.concatenate([qn[HD // 2 :], qn[: HD // 2]])
    kn_rot = np.concatenate([kn[HD // 2 :], kn[: HD // 2]])
    sgn = np.concatenate(
        [-np.ones(HD // 2, np.float32), np.ones(HD // 2, np.float32)]
    )
    cosq_t = cos * qn[None, :]
    sinq_t = sin * (qn_rot * sgn)[None, :]
    cosk_t = cos * kn[None, :]
    sink_t = sin * (kn_rot * sgn)[None, :]

    wt = {
        "q": np.asarray(wq, np.float32).T,  # [H, OQ]
        "k": np.asarray(wk, np.float32).T,
        "v": np.asarray(wv, np.float32).T,
        "o": np.asarray(wo, np.float32).T,  # [H(=in), OO]
    }
    worder = ("k", "v", "q", "o")
    wconst = np.concatenate(
        [
            np.array([WNUMEL[m] for m in worder], np.float32),
            np.array([1.0 / WNUMEL[m] for m in worder], np.float32),
        ]
    ).reshape(1, 8)

    p = np.arange(128)[:, None]
    f = np.arange(128)[None, :]
    tri = (p <= f)  # pel[k, q] upper-incl triangle within the diagonal tile

    in_maps = []
    for c in range(NC):
        b, z = c // GROUP, c % GROUP
        t0a = b * S + z * 128  # q-tile0 = batch tile z
        t0b = b * S + (7 - z) * 128  # q-tile1 = batch tile 7-z
        rows = np.r_[t0a : t0a + 128, t0b : t0b + 128]
        # masks: [128 k, 2, 4, 128 q]
        mask = np.zeros((128, 2, NJ0, 128), np.float32)
        for j in range(NJ0):  # q-tile0 (tile z) vs key tiles 0..3
            if j < z:
                mask[:, 0, j, :] = 1.0
            elif j == z:
                mask[:, 0, j, :] = tri
        for j in range(NJ0, KTILES):  # q-tile1 (tile 7-z) vs key tiles 4..7
            if j < 7 - z:
                mask[:, 1, j - NJ0, :] = 1.0
            elif j == 7 - z:
                mask[:, 1, j - NJ0, :] = tri
        pos = np.r_[z * 128 : z * 128 + 128, (7 - z) * 128 : (8 - z) * 128]
        m = {
            "x_sl": np.ascontiguousarray(x2[rows]),
            "cosq": np.ascontiguousarray(cosq_t[pos]),
            "sinq": np.ascontiguousarray(sinq_t[pos]),
            "cosk": np.ascontiguousarray(cosk_t[pos]),
            "sink": np.ascontiguousarray(sink_t[pos]),
            "wq_sl": np.ascontiguousarray(wt["q"][c * HSL : (c + 1) * HSL]),
            "wk_sl": np.ascontiguousarray(wt["k"][c * HSL : (c + 1) * HSL]),
            "wv_sl": np.ascontiguousarray(wt["v"][c * HSL : (c + 1) * HSL]),
            "wo_sl": np.ascontiguousarray(wt["o"][c * HSL : (c + 1) * HSL]),
            "mask": mask.astype(ml_dtypes.bfloat16),
            "wconst": wconst,
        }
        in_maps.append(m)
    return in_maps


def kernel(x, cos, sin, wq, wk, wv, wo, qn, kn):
    if "nc" not in _CACHE:
        _CACHE["nc"] = _build()
    nc = _CACHE["nc"]
    in_maps = _host_inputs(x, cos, sin, wq, wk, wv, wo, qn, kn)
    res = bass_utils.run_bass_kernel_spmd(nc, in_maps, core_ids=list(range(NC)))
    full = np.zeros((B * S, H), np.float32)
    for c in range(NC):
        b, z = c // GROUP, c % GROUP
        o = np.asarray(res.results[c]["out"])
        t0a = b * S + z * 128
        t0b = b * S + (7 - z) * 128
        full[t0a : t0a + 128] = o[0:128]
        full[t0b : t0b + 128] = o[128:256]
    return full.reshape(B, S, H)


# revision 25
# speedup vs baseline: 1.1009x; 1.0587x over previous
"""BitNet GQA attention layer on 8 TRN2 NeuronCores — v4.

Sharding: token-parallel with zigzag causal balance. B*S = 2048 tokens ->
256 per core. Core c (batch b=c//4, zig z=c%4) owns query tiles {z, 7-z}
of batch b, so every core computes the same 12 score blocks per head
(4 for tile z masked per-core, 8 for tile 7-z) instead of the naive 20.

Weights are split 8-way along the contraction dim for quantization (one
tiny AllReduce for all four matrices' abs-sums), then ternary fp8
weights are AllGathered (k+v merged, then q; o last, explicitly ordered
after the kv-activation gather so it can't jump the CC queue). K/V
activations are AllGathered within each batch's 4-core group (k and v
merged into one collective). Transposes run on the PE (fp32 for the
integer activations with the round-bias fold, bf16 for rope outputs).
BitNet matmuls are exact integer arithmetic in bf16 x fp8 with fp32
PSUM accumulation; projection loops run hi-outer/chunk-inner with
contiguous 128-col stationary slices.
"""

import sys

sys.path.insert(0, "/opt/trn_rl_repo")

import numpy as np
import ml_dtypes

import concourse.bass as bass
import concourse.mybir as mybir
import concourse.tile as tile
from concourse import bacc
from concourse import bass_utils
from concourse.masks import make_identity

F32 = mybir.dt.float32
BF16 = mybir.dt.bfloat16
FP8 = mybir.dt.float8e4
AX = mybir.AxisListType.X
OP = mybir.AluOpType
AF = mybir.ActivationFunctionType

B, S, H = 2, 1024, 2048
NH, NKV, HD = 16, 8, 128
NC = 8
T = (B * S) // NC  # 256 tokens per core
TB = T // 128  # 2 token tiles per core
HSL = H // NC  # 256 weight rows per core
EPS = 1e-6
RND = 12582912.0  # 1.5 * 2**23: fp32 add => round-to-nearest-even
INV_SQRT_HD = 1.0 / float(np.sqrt(HD))
KTILES = S // 128  # 8 key tiles per batch
GROUP = 4  # cores per batch
NJ0 = 4  # key slots computed for q-tile0 (covers z <= 3)
NJ1 = KTILES  # key slots computed for q-tile1

OQ, OK, OV, OO = H, NKV * HD, NKV * HD, H  # 2048, 1024, 1024, 2048
OW = {"q": OQ, "k": OK, "v": OV, "o": OO}
WNUMEL = {m: OW[m] * H for m in OW}
HI_N = H // 128  # 16 contraction tiles

_CACHE = {}


def _build():
    nc = bacc.Bacc("TRN2", target_bir_lowering=False, debug=False, num_devices=NC)

    x_sl = nc.dram_tensor("x_sl", [T, H], F32, kind="ExternalInput")
    cosq = nc.dram_tensor("cosq", [T, HD], F32, kind="ExternalInput")
    sinq = nc.dram_tensor("sinq", [T, HD], F32, kind="ExternalInput")
    cosk = nc.dram_tensor("cosk", [T, HD], F32, kind="ExternalInput")
    sink = nc.dram_tensor("sink", [T, HD], F32, kind="ExternalInput")
    w_sl = {
        "q": nc.dram_tensor("wq_sl", [HSL, OQ], F32, kind="ExternalInput"),
        "k": nc.dram_tensor("wk_sl", [HSL, OK], F32, kind="ExternalInput"),
        "v": nc.dram_tensor("wv_sl", [HSL, OV], F32, kind="ExternalInput"),
        "o": nc.dram_tensor("wo_sl", [HSL, OO], F32, kind="ExternalInput"),
    }
    # mask[p, a, j, f]: a=0 -> pel[k=p, slot j (abs key tile j), q=f] of tile z
    #                  a=1 -> slots 4..7 (abs key tiles 4..7) of tile 7-z
    mask_in = nc.dram_tensor("mask", [128, 2, NJ0, 128], BF16, kind="ExternalInput")
    # cols 0-3: numel for k,v,q,o ; cols 4-7: 1/numel for k,v,q,o
    wconst = nc.dram_tensor("wconst", [1, 8], F32, kind="ExternalInput")
    out = nc.dram_tensor("out", [T, H], F32, kind="ExternalOutput")

    with tile.TileContext(nc) as tc:
        _build_body(nc, tc, x_sl, cosq, sinq, cosk, sink, w_sl, mask_in, wconst, out)

    nc.compile()
    return nc


def _build_body(nc, tc, x_sl, cosq, sinq, cosk, sink, w_sl, mask_in, wconst, out):
    sync = nc.sync

    with (
        tc.tile_pool(name="dram", bufs=1, space="DRAM") as dram,
        tc.tile_pool(name="const", bufs=1) as constp,
        tc.tile_pool(name="vecs", bufs=1) as vecs,
        tc.tile_pool(name="persist", bufs=1) as persist,
        tc.tile_pool(name="ptrans", bufs=2, space="PSUM") as ptrans,
    ):
        # ---- DRAM bounce buffers for collectives ----
        wag_kv = dram.tile([HSL, OK + OV], FP8)
        wint_kv = dram.tile([H, OK + OV], FP8, addr_space="Shared")
        wag_q = dram.tile([HSL, OQ], FP8)
        wint_q = dram.tile([H, OQ], FP8, addr_space="Shared")
        wag_o = dram.tile([HSL, OO], FP8)
        wint_o = dram.tile([H, OO], FP8, addr_space="Shared")
        ar_in = dram.tile([1, 8], F32)
        ar_out = dram.tile([1, 8], F32, addr_space="Shared")
        # cols 0:2048 = kT (hk,t); cols 2048:4096 = v (a,hk,d)
        kv_in = dram.tile([128, NKV * T + TB * NKV * HD], BF16)
        kv_out = dram.tile([512, NKV * T + TB * NKV * HD], BF16)

        # ---- constants ----
        ones1 = constp.tile([1, 128], F32)
        nc.vector.memset(ones1, 1.0)
        onescol = constp.tile([128, 1], F32)
        nc.vector.memset(onescol, 1.0)
        wconst_sb = constp.tile([1, 8], F32)
        sync.dma_start(wconst_sb, wconst.ap())
        negrnd = constp.tile([128, 1], F32)
        nc.vector.memset(negrnd, -RND)
        epsb = constp.tile([128, 1], F32)
        nc.vector.memset(epsb, EPS)
        ident = constp.tile([128, 128], F32)
        make_identity(nc, ident)
        identb = constp.tile([128, 128], BF16)
        make_identity(nc, identb)
        mask_sb = constp.tile([128, 2, NJ0, 128], BF16)
        sync.dma_start(mask_sb, mask_in.ap())
        cs = {}
        for nm, t in (("cq", cosq), ("sq", sinq), ("ck", cosk), ("sk", sink)):
            c = constp.tile([128, TB, HD], F32, name=f"cs_{nm}")
            sync.dma_start(c, t.ap().rearrange("(a p) d -> p a d", p=128))
            cs[nm] = c

        # persistent activations (stationary slices contiguous: [.., tb, hi, 128])
        xqT = persist.tile([128, TB, HI_N, 128], BF16)
        qT = persist.tile([128, NH, T], BF16)  # [d, head, t]
        kT = persist.tile([128, NKV, T], BF16)  # [d, kv head, t] (local)
        v_loc = persist.tile([128, TB, NKV, HD], BF16)
        q_sb = persist.tile([128, TB, OQ], F32)
        k_sb = persist.tile([128, TB, OK], F32)
        attn = persist.tile([128, TB, H], F32)
        aT = persist.tile([128, TB, HI_N, 128], BF16)
        # gathered k/v keyed by (group-core z, slot a); key tile j = (zj, aj)
        kT_all = persist.tile([128, NKV, GROUP, TB, 128], BF16)
        v_all = persist.tile([128, GROUP, TB, NKV, 130], BF16)

        # ====== Phase W: x quant/transpose + weight scales + quant + AGs ======
        dqx = []
        with (
            tc.tile_pool(name="xraw", bufs=2) as xraw,
            tc.tile_pool(name="wraw_k", bufs=1) as wraw_k,
            tc.tile_pool(name="wraw_v", bufs=1) as wraw_v,
            tc.tile_pool(name="wraw_q", bufs=1) as wraw_q,
            tc.tile_pool(name="wraw_o", bufs=1) as wraw_o,
            tc.tile_pool(name="scr", bufs=2) as scr,
            tc.tile_pool(name="wtmp", bufs=1) as wtmp,
            tc.tile_pool(name="wq8", bufs=2) as wq8,
            tc.tile_pool(name="psmall", bufs=2, space="PSUM") as psmall,
        ):
            # ---- x: load + act_quant + PE transpose ----
            xs_t = []
            for tb in range(TB):
                xs = xraw.tile([128, H], F32, tag="xs", name=f"xs{tb}")
                sync.dma_start(xs, x_sl.ap()[tb * 128 : (tb + 1) * 128, :])
                xs_t.append(xs)

            wraws = {"k": wraw_k, "v": wraw_v, "q": wraw_q, "o": wraw_o}
            worder = ("k", "v", "q", "o")
            wab = {}
            for m in worder:
                for pt in range(2):
                    wr = wraws[m].tile([128, OW[m]], F32, name=f"wr_{m}{pt}")
                    sync.dma_start(wr, w_sl[m].ap()[pt * 128 : (pt + 1) * 128, :])
                    wab[(m, pt)] = wr

            for tb in range(TB):
                xs = xs_t[tb]
                axm = vecs.tile([128, 1], F32, name=f"axm{tb}")
                nc.vector.tensor_reduce(
                    axm, xs, axis=AX, op=OP.max, apply_absolute_value=True
                )
                rsx = vecs.tile([128, 1], F32, name=f"rsx{tb}")
                nc.vector.reciprocal(rsx, axm)
                sxq = vecs.tile([128, 1], F32, name=f"sxq{tb}")
                nc.vector.tensor_scalar_mul(sxq, rsx, 127.0)
                dq = vecs.tile([128, 1], F32, name=f"dqx{tb}")
                nc.vector.tensor_scalar_mul(dq, axm, 1.0 / 127.0)
                dqx.append(dq)
                nc.vector.tensor_scalar(
                    xs, xs, sxq, RND, op0=OP.mult, op1=OP.add
                )
                for hg in range(0, HI_N, 4):
                    pt4 = ptrans.tile([128, 4, 128], F32, tag="ptr")
                    for i in range(4):
                        hi = hg + i
                        nc.tensor.transpose(
                            pt4[:, i, :], xs[:, hi * 128 : (hi + 1) * 128], ident
                        )
                    nc.scalar.activation(
                        xqT[:, tb, hg : hg + 4, :], pt4, AF.Identity, bias=negrnd
                    )

            # ---- weight abs-sums on the scalar engine (accumulate output) ----
            red0 = vecs.tile([128, 4], F32, name="red0")
            red1 = vecs.tile([128, 4], F32, name="red1")
            for mi, m in enumerate(worder):
                for pt, red in ((0, red0), (1, red1)):
                    sc = scr.tile([128, OW[m]], F32, tag="scr", name=f"sc_{m}{pt}")
                    nc.scalar.activation(
                        sc, wab[(m, pt)], AF.Abs, accum_out=red[:, mi : mi + 1]
                    )
            redc = vecs.tile([128, 4], F32, name="redc")
            nc.vector.tensor_add(redc, red0, red1)
            ps = psmall.tile([1, 4], F32, name="ps_sums", tag="psm")
            nc.tensor.matmul(ps, onescol, redc, start=True, stop=True)
            sums = vecs.tile([1, 8], F32, name="sums")
            nc.vector.memset(sums, 0.0)
            nc.scalar.copy(sums[:, 0:4], ps)
            nc.scalar.dma_start(ar_in, sums)
            nc.gpsimd.collective_compute(
                "AllReduce", OP.add, replica_groups=[list(range(NC))],
                ins=[ar_in.opt()], outs=[ar_out.opt()],
            )

            # ---- scales from the AllReduce ----
            g = vecs.tile([1, 8], F32, name="g")
            nc.scalar.dma_start(g, ar_out)
            r4 = vecs.tile([1, 4], F32, name="r4")
            nc.vector.reciprocal(r4, g[:, 0:4])
            sw8 = vecs.tile([1, 8], F32, name="sw8")
            nc.vector.tensor_mul(sw8[:, 0:4], r4, wconst_sb[:, 0:4])
            nc.vector.tensor_mul(sw8[:, 4:8], g[:, 0:4], wconst_sb[:, 4:8])
            pb = psmall.tile([128, 8], F32, name="pb", tag="psm")
            nc.tensor.matmul(pb, ones1, sw8, start=True, stop=True)
            sb8 = vecs.tile([128, 8], F32, name="sb8")
            nc.scalar.copy(sb8, pb)
            rswb = {m: sb8[:, 4 + mi : 5 + mi] for mi, m in enumerate(worder)}

            def w_quant(m, mi, dst, col0):
                for pt in range(2):
                    wr = wab[(m, pt)]
                    t1 = wtmp.tile([128, OW[m]], F32, tag="wtmp", name=f"t1_{m}{pt}")
                    nc.vector.tensor_scalar(
                        t1, wr, sb8[:, mi : mi + 1], RND, op0=OP.mult, op1=OP.add
                    )
                    nc.scalar.activation(t1, t1, AF.Identity, bias=negrnd)
                    wi = wq8.tile([128, OW[m]], FP8, tag="wi")
                    nc.vector.tensor_scalar(wi, t1, 1.0, -1.0, op0=OP.min, op1=OP.max)
                    nc.scalar.dma_start(
                        dst[pt * 128 : (pt + 1) * 128, col0 : col0 + OW[m]], wi
                    )

            w_quant("k", 0, wag_kv, 0)
            w_quant("v", 1, wag_kv, OK)
            nc.gpsimd.collective_compute(
                "AllGather", OP.bypass, replica_groups=[list(range(NC))],
                ins=[wag_kv.opt()], outs=[wint_kv.opt()],
            )
            w_quant("q", 2, wag_q, 0)
            nc.gpsimd.collective_compute(
                "AllGather", OP.bypass, replica_groups=[list(range(NC))],
                ins=[wag_q.opt()], outs=[wint_q.opt()],
            )
            w_quant("o", 3, wag_o, 0)

        # dequant vectors (absmax/127 * 1/s_w)
        dqv = {}
        for m in ("q", "k", "v", "o"):
            for tb in range(TB):
                d = vecs.tile([128, 1], F32, name=f"dqv_{m}{tb}")
                nc.vector.tensor_mul(d, dqx[tb], rswb[m])
                dqv[(m, tb)] = d

        def rope_batch(src_sb, tb, nh, cosn, sinn, dstT, ropep, label):
            w = nh * 128
            blk = src_sb[:, tb, :]  # [128, w] f32
            sq = ropep.tile([128, w], F32, tag="unf", padded_shape=[128, NH * 128])
            nc.scalar.activation(sq, blk, AF.Square)
            ms = vecs.tile([128, nh], F32, name=f"ms_{label}{tb}")
            nc.vector.tensor_reduce(
                ms, sq.rearrange("p (h d) -> p h d", h=nh), axis=AX, op=OP.add
            )
            rms = vecs.tile([128, nh], F32, name=f"rms_{label}{tb}")
            nc.scalar.activation(rms, ms, AF.Sqrt, scale=1.0 / HD, bias=epsb)
            rn = vecs.tile([128, nh], F32, name=f"rn_{label}{tb}")
            nc.vector.reciprocal(rn, rms)
            rnb = rn.to_broadcast([128, nh, 128])
            blk3 = blk.rearrange("p (h d) -> p h d", h=nh)
            un2 = ropep.tile(
                [128, nh * 128], F32, tag="unf", padded_shape=[128, NH * 128],
                name="un2",
            )
            un = un2.rearrange("p (h d) -> p h d", h=nh)
            nc.vector.tensor_mul(un, blk3, rnb)
            cosb = (
                cs[cosn][:, tb, :]
                .rearrange("p (one d) -> p one d", one=1)
                .to_broadcast([128, nh, 128])
            )
            sinb = (
                cs[sinn][:, tb, :]
                .rearrange("p (one d) -> p one d", one=1)
                .to_broadcast([128, nh, 128])
            )
            ra2 = ropep.tile([128, nh * 128], F32, tag="ra", padded_shape=[128, NH * 128])
            ra = ra2.rearrange("p (h d) -> p h d", h=nh)
            nc.vector.tensor_mul(ra, un, cosb)
            rb2 = ropep.tile([128, nh * 128], F32, tag="rb", padded_shape=[128, NH * 128])
            rb = rb2.rearrange("p (h d) -> p h d", h=nh)
            nc.vector.tensor_mul(rb[:, :, 0:64], un[:, :, 64:128], sinb[:, :, 0:64])
            nc.vector.tensor_mul(rb[:, :, 64:128], un[:, :, 0:64], sinb[:, :, 64:128])
            raf = ropep.tile(
                [128, nh * 128], BF16, tag="raf", padded_shape=[128, NH * 128]
            )
            nc.vector.tensor_add(raf, ra2, rb2)
            for hg in range(0, nh, 4):
                pt4 = ptrans.tile(
                    [128, 4, 128], BF16, tag="ptr", padded_shape=[128, 4, 256]
                )
                for i in range(4):
                    nc.tensor.transpose(
                        pt4[:, i, :], raf[:, (hg + i) * 128 : (hg + i + 1) * 128],
                        identb,
                    )
                nc.scalar.activation(
                    dstT[:, hg : hg + 4, tb * 128 : (tb + 1) * 128], pt4, AF.Copy
                )

        # ====== Phase KV/Q: projections (hi-outer, chunk-inner) ======
        def proj_load(wint_src, o_w, m, wpool):
            src3 = wint_src.rearrange("(hi p) o -> p hi o", p=128)
            chunks = []
            for cg in range(4):
                wst = wpool.tile(
                    [128, 4, o_w], FP8, tag="wst",
                    padded_shape=[128, 4, OK + OV], name=f"wst_{m}{cg}",
                )
                sync.dma_start(wst, src3[:, cg * 4 : (cg + 1) * 4, :])
                chunks.append(wst)
            return chunks

        def proj_tb(chunks, o_w, tb, ppool, evac):
            """x.T @ w for one token tile; evac(oc, psum_tile) per 512-chunk."""
            ncols = o_w // 512
            pps = [
                ppool.tile([128, 512], F32, tag="pp", name=f"pp{tb}_{i}")
                for i in range(ncols)
            ]
            for hi in range(HI_N):
                xsl = xqT[:, tb, hi, :]
                for oc in range(ncols):
                    nc.tensor.matmul(
                        pps[oc],
                        xsl,
                        chunks[hi // 4][:, hi % 4, oc * 512 : (oc + 1) * 512],
                        start=(hi == 0),
                        stop=(hi == HI_N - 1),
                    )
            for oc in range(ncols):
                evac(oc, pps[oc])

        with (
            tc.tile_pool(name="wmm1", bufs=5) as wmm1,
            tc.tile_pool(name="pproj", bufs=6, space="PSUM") as pproj,
            tc.tile_pool(name="ropep", bufs=1) as ropep,
        ):
            wst_kv = proj_load(wint_kv, OK + OV, "kv", wmm1)

            def evac_kv(tb):
                def f(oc, pp):
                    if oc < 2:  # k chunks
                        nc.vector.tensor_scalar(
                            k_sb[:, tb, oc * 512 : (oc + 1) * 512],
                            pp, dqv[("k", tb)], None, op0=OP.mult,
                        )
                    else:  # v chunks
                        nc.vector.tensor_scalar(
                            v_loc[:, tb, (oc - 2) * 4 : (oc - 1) * 4, :],
                            pp, dqv[("v", tb)], None, op0=OP.mult,
                        )
                return f

            for tb in range(TB):
                proj_tb(wst_kv, OK + OV, tb, pproj, evac_kv(tb))
                rope_batch(k_sb, tb, NKV, "ck", "sk", kT, ropep, "k")

            # bounce k/v to DRAM and allgather within the batch group
            nc.scalar.dma_start(
                kv_in[:, 0 : NKV * T].rearrange("p (hk t) -> p hk t", hk=NKV), kT
            )
            nc.scalar.dma_start(
                kv_in[:, NKV * T :].rearrange(
                    "p (a hk d) -> p a hk d", a=TB, hk=NKV
                ),
                v_loc,
            )
            cc_kv = nc.gpsimd.collective_compute(
                "AllGather", OP.bypass,
                replica_groups=[[0, 1, 2, 3], [4, 5, 6, 7]],
                ins=[kv_in.opt()], outs=[kv_out.opt()],
            )

            # ---- Q projection + rope (overlaps the KV allgather) ----
            wst_q = proj_load(wint_q, OQ, "q", wmm1)

            def evac_q(tb):
                def f(oc, pp):
                    nc.vector.tensor_scalar(
                        q_sb[:, tb, oc * 512 : (oc + 1) * 512],
                        pp, dqv[("q", tb)], None, op0=OP.mult,
                    )
                return f

            for tb in range(TB):
                proj_tb(wst_q, OQ, tb, pproj, evac_q(tb))
                rope_batch(q_sb, tb, NH, "cq", "sq", qT, ropep, "q")
            cc_o = nc.gpsimd.collective_compute(
                "AllGather", OP.bypass, replica_groups=[list(range(NC))],
                ins=[wag_o.opt()], outs=[wint_o.opt()],
            )
            # keep the o-weight gather off the critical kv-activation gather
            bass._add_dep_helper(
                cc_o.ins, cc_kv.ins, sync=True,
                reason="order wo allgather after kv allgather",
            )

        # ====== Phase A: attention ======
        with tc.tile_pool(name="wmm2", bufs=1) as wmm2:
            # prefetch o_proj weights under the attention phase
            src3o = wint_o.rearrange("(hi p) o -> p hi o", p=128)
            wsto = wmm2.tile([128, HI_N, OO], FP8, tag="wst2")
            sync.dma_start(wsto, src3o)

            # gather readback, keyed by (group-core z, slot a)
            KL = NKV * T  # k part length in kv_out cols
            for zc in range(GROUP):
                rows = kv_out[128 * zc : 128 * zc + 128, :]
                src_k = rows[:, 0:KL].rearrange("d (hk a t) -> d hk a t", hk=NKV, a=TB)
                nc.scalar.dma_start(kT_all[:, :, zc, :, :], src_k)
                src_v = rows[:, KL:].rearrange(
                    "p (a hk d) -> p a hk d", a=TB, hk=NKV
                )
                nc.scalar.dma_start(v_all[:, zc, :, :, 0:128], src_v)
            nc.vector.memset(v_all[:, :, :, :, 128:130], 1.0)
            # key tile j lives at (zj, aj)
            JZA = [(j, 0) if j < GROUP else (7 - j, 1) for j in range(KTILES)]

            with (
                tc.tile_pool(name="pscore", bufs=2, space="PSUM") as pscore,
                tc.tile_pool(name="ppv", bufs=2, space="PSUM") as ppv,
                tc.tile_pool(name="pexp", bufs=3) as pexp,
            ):
                pels = {}

                def scores(h):
                    hk = h // 2
                    pel = pexp.tile([128, KTILES, T], BF16, tag="pel")
                    pels[h] = pel
                    # slots 0..3: both q-tiles (N=256); slots 4..7: q-tile1 only
                    for g in range(2):
                        st = pscore.tile([128, 2, T], F32, tag="st")
                        for i in range(2):
                            zj, aj = JZA[2 * g + i]
                            nc.tensor.matmul(
                                st[:, i, :], kT_all[:, hk, zj, aj, :], qT[:, h, :],
                                start=True, stop=True,
                            )
                        nc.scalar.activation(
                            pel[:, 2 * g : 2 * g + 2, :], st, AF.Exp,
                            scale=INV_SQRT_HD,
                        )
                    st2 = pscore.tile([128, 4, 128], F32, tag="st2")
                    for i in range(4):
                        zj, aj = JZA[4 + i]
                        nc.tensor.matmul(
                            st2[:, i, :],
                            kT_all[:, hk, zj, aj, :], qT[:, h, 128:256],
                            start=True, stop=True,
                        )
                    nc.scalar.activation(
                        pel[:, NJ0:KTILES, 128:256], st2, AF.Exp,
                        scale=INV_SQRT_HD,
                    )
                    # per-core causal masks: q-tile0 all 4 slots; q-tile1 slots 4-7
                    nc.vector.tensor_mul(
                        pel[:, 0:NJ0, 0:128], pel[:, 0:NJ0, 0:128], mask_sb[:, 0, :, :]
                    )
                    nc.vector.tensor_mul(
                        pel[:, NJ0:KTILES, 128:256],
                        pel[:, NJ0:KTILES, 128:256],
                        mask_sb[:, 1, :, :],
                    )

                omax = vecs.tile([128, TB], F32, name="omax")

                def pv(h):
                    hk = h // 2
                    pel = pels.pop(h)
                    for a, nj in ((0, NJ0), (1, NJ1)):
                        po = ppv.tile([128, 132], F32, tag="po")
                        for j in range(nj):
                            zj, aj = JZA[j]
                            nc.tensor.matmul(
                                po[:, 0:129],
                                pel[:, j, a * 128 : (a + 1) * 128],
                                v_all[:, zj, aj, hk, 0:129],
                                start=(j == 0),
                                stop=(j == nj - 1),
                            )
                        rden = vecs.tile([128, 1], F32, name=f"rden{h}_{a}")
                        nc.vector.reciprocal(rden, po[:, 128:129])
                        nc.vector.tensor_scalar(
                            attn[:, a, h * 128 : (h + 1) * 128],
                            po[:, 0:128], rden, None, op0=OP.mult,
                        )
                    # running |attn| max per token tile (feeds o act_quant)
                    oax = vecs.tile([128, TB], F32, name=f"oax{h}")
                    nc.vector.tensor_reduce(
                        oax, attn[:, :, h * 128 : (h + 1) * 128], axis=AX,
                        op=OP.max, apply_absolute_value=True,
                    )
                    if h == 0:
                        nc.vector.tensor_copy(omax, oax)
                    else:
                        nc.vector.tensor_max(omax, omax, oax)

                # software-pipelined: scores one head ahead of PV
                scores(0)
                for h in range(NH):
                    if h + 1 < NH:
                        scores(h + 1)
                    pv(h)

            # ====== Phase O: act_quant(attn) + o_proj ======
            with (
                tc.tile_pool(name="oq", bufs=2) as oq,
                tc.tile_pool(name="pproj2", bufs=4, space="PSUM") as pproj2,
                tc.tile_pool(name="osb", bufs=2) as osb,
            ):
                dqo = []
                for tb in range(TB):
                    axm = omax[:, tb : tb + 1]
                    rsx = vecs.tile([128, 1], F32, name=f"orsx{tb}")
                    nc.vector.reciprocal(rsx, axm)
                    sxq = vecs.tile([128, 1], F32, name=f"osxq{tb}")
                    nc.vector.tensor_scalar_mul(sxq, rsx, 127.0)
                    dq = vecs.tile([128, 1], F32, name=f"odqx{tb}")
                    nc.vector.tensor_scalar_mul(dq, axm, 1.0 / 127.0)
                    d2 = vecs.tile([128, 1], F32, name=f"odq2{tb}")
                    nc.vector.tensor_mul(d2, dq, rswb["o"])
                    dqo.append(d2)
                    ar = oq.tile([128, H], F32, tag="ar")
                    nc.vector.tensor_scalar(
                        ar, attn[:, tb, :], sxq, RND, op0=OP.mult, op1=OP.add
                    )
                    for hg in range(0, HI_N, 4):
                        pt4 = ptrans.tile([128, 4, 128], F32, tag="ptr")
                        for i in range(4):
                            hi = hg + i
                            nc.tensor.transpose(
                                pt4[:, i, :], ar[:, hi * 128 : (hi + 1) * 128], ident
                            )
                        nc.scalar.activation(
                            aT[:, tb, hg : hg + 4, :], pt4, AF.Identity, bias=negrnd
                        )

                for tb in range(TB):
                    pps = [
                        pproj2.tile([128, 512], F32, tag="pp2", name=f"pp2_{tb}_{i}")
                        for i in range(4)
                    ]
                    for hi in range(HI_N):
                        asl = aT[:, tb, hi, :]
                        for oc in range(4):
                            nc.tensor.matmul(
                                pps[oc],
                                asl,
                                wsto[:, hi, oc * 512 : (oc + 1) * 512],
                                start=(hi == 0),
                                stop=(hi == HI_N - 1),
                            )
                    for oc in range(4):
                        ot = osb.tile([128, 512], F32, tag="ot")
                        nc.vector.tensor_scalar(ot, pps[oc], dqo[tb], None, op0=OP.mult)
                        sync.dma_start(
                            out.ap()[
                                tb * 128 : (tb + 1) * 128, oc * 512 : (oc + 1) * 512
                            ],
                            ot,
                        )


def _host_inputs(x, cos, sin, wq, wk, wv, wo, qn, kn):
    """Build the 8 per-core input maps (pure slicing / layout transforms)."""
    x2 = np.asarray(x, np.float32).reshape(B * S, H)
    cos = np.asarray(cos, np.float32)
    sin = np.asarray(sin, np.float32)
    qn = np.asarray(qn, np.float32)
    kn = np.asarray(kn, np.float32)
    # fold qk-norm weights into rope tables (exact identity when qn=kn=1)
    qn_rot = np.concatenate([qn[HD // 2 :], qn[: HD // 2]])
    kn_rot = np.concatenate([kn[HD // 2 :], kn[: HD // 2]])
    sgn = np.concatenate(
        [-np.ones(HD // 2, np.float32), np.ones(HD // 2, np.float32)]
    )
    cosq_t = cos * qn[None, :]
    sinq_t = sin * (qn_rot * sgn)[None, :]
    cosk_t = cos * kn[None, :]
    sink_t = sin * (kn_rot * sgn)[None, :]

    wt = {
        "q": np.asarray(wq, np.float32).T,  # [H, OQ]
        "k": np.asarray(wk, np.float32).T,
        "v": np.asarray(wv, np.float32).T,
        "o": np.asarray(wo, np.float32).T,  # [H(=in), OO]
    }
    worder = ("k", "v", "q", "o")
    wconst = np.concatenate(
        [
            np.array([WNUMEL[m] for m in worder], np.float32),
            np.array([1.0 / WNUMEL[m] for m in worder], np.float32),
        ]
    ).reshape(1, 8)

    p = np.arange(128)[:, None]
    f = np.arange(128)[None, :]
    tri = (p <= f)  # pel[k, q] upper-incl triangle within the diagonal tile

    in_maps = []
    for c in range(NC):
        b, z = c // GROUP, c % GROUP
        t0a = b * S + z * 128  # q-tile0 = batch tile z
        t0b = b * S + (7 - z) * 128  # q-tile1 = batch tile 7-z
        rows = np.r_[t0a : t0a + 128, t0b : t0b + 128]
        # masks: [128 k, 2, 4, 128 q]
        mask = np.zeros((128, 2, NJ0, 128), np.float32)
        for j in range(NJ0):  # q-tile0 (tile z) vs key tiles 0..3
            if j < z:
                mask[:, 0, j, :] = 1.0
            elif j == z:
                mask[:, 0, j, :] = tri
        for j in range(NJ0, KTILES):  # q-tile1 (tile 7-z) vs key tiles 4..7
            if j < 7 - z:
                mask[:, 1, j - NJ0, :] = 1.0
            elif j == 7 - z:
                mask[:, 1, j - NJ0, :] = tri
        pos = np.r_[z * 128 : z * 128 + 128, (7 - z) * 128 : (8 - z) * 128]
        m = {
            "x_sl": np.ascontiguousarray(x2[rows]),
            "cosq": np.ascontiguousarray(cosq_t[pos]),
            "sinq": np.ascontiguousarray(sinq_t[pos]),
            "cosk": np.ascontiguousarray(cosk_t[pos]),
            "sink": np.ascontiguousarray(sink_t[pos]),
            "wq_sl": np.ascontiguousarray(wt["q"][c * HSL : (c + 1) * HSL]),
            "wk_sl": np.ascontiguousarray(wt["k"][c * HSL : (c + 1) * HSL]),
            "wv_sl": np.ascontiguousarray(wt["v"][c * HSL : (c + 1) * HSL]),
            "wo_sl": np.ascontiguousarray(wt["o"][c * HSL : (c + 1) * HSL]),
            "mask": mask.astype(ml_dtypes.bfloat16),
            "wconst": wconst,
        }
        in_maps.append(m)
    return in_maps


def kernel(x, cos, sin, wq, wk, wv, wo, qn, kn):
    if "nc" not in _CACHE:
        _CACHE["nc"] = _build()
    nc = _CACHE["nc"]
    in_maps = _host_inputs(x, cos, sin, wq, wk, wv, wo, qn, kn)
    res = bass_utils.run_bass_kernel_spmd(nc, in_maps, core_ids=list(range(NC)))
    full = np.zeros((B * S, H), np.float32)
    for c in range(NC):
        b, z = c // GROUP, c % GROUP
        o = np.asarray(res.results[c]["out"])
        t0a = b * S + z * 128
        t0b = b * S + (7 - z) * 128
        full[t0a : t0a + 128] = o[0:128]
        full[t0b : t0b + 128] = o[128:256]
    return full.reshape(B, S, H)


# revision 29
# speedup vs baseline: 1.1849x; 1.0763x over previous
"""BitNet GQA attention layer on 8 TRN2 NeuronCores — v4.

Sharding: token-parallel with zigzag causal balance. B*S = 2048 tokens ->
256 per core. Core c (batch b=c//4, zig z=c%4) owns query tiles {z, 7-z}
of batch b, so every core computes the same 12 score blocks per head
(4 for tile z masked per-core, 8 for tile 7-z) instead of the naive 20.

Weights are split 8-way along the contraction dim for quantization (one
tiny AllReduce for all four matrices' abs-sums), then ternary fp8
weights are AllGathered (k+v merged, then q; o last, explicitly ordered
after the kv-activation gather so it can't jump the CC queue). K/V
activations are AllGathered within each batch's 4-core group (k and v
merged into one collective). Transposes run on the PE (fp32 for the
integer activations with the round-bias fold, bf16 for rope outputs).
BitNet matmuls are exact integer arithmetic in bf16 x fp8 with fp32
PSUM accumulation; projection loops run hi-outer/chunk-inner with
contiguous 128-col stationary slices.
"""

import sys

sys.path.insert(0, "/opt/trn_rl_repo")

import numpy as np
import ml_dtypes

import concourse.bass as bass
import concourse.mybir as mybir
import concourse.tile as tile
from concourse import bacc
from concourse import bass_utils
from concourse.masks import make_identity

F32 = mybir.dt.float32
BF16 = mybir.dt.bfloat16
FP8 = mybir.dt.float8e4
AX = mybir.AxisListType.X
OP = mybir.AluOpType
AF = mybir.ActivationFunctionType

B, S, H = 2, 1024, 2048
NH, NKV, HD = 16, 8, 128
NC = 8
T = (B * S) // NC  # 256 tokens per core
TB = T // 128  # 2 token tiles per core
HSL = H // NC  # 256 weight rows per core
EPS = 1e-6
RND = 12582912.0  # 1.5 * 2**23: fp32 add => round-to-nearest-even
INV_SQRT_HD = 1.0 / float(np.sqrt(HD))
KTILES = S // 128  # 8 key tiles per batch
GROUP = 4  # cores per batch
NJ0 = 4  # key slots computed for q-tile0 (covers z <= 3)
NJ1 = KTILES  # key slots computed for q-tile1

OQ, OK, OV, OO = H, NKV * HD, NKV * HD, H  # 2048, 1024, 1024, 2048
OW = {"q": OQ, "k": OK, "v": OV, "o": OO}
WNUMEL = {m: OW[m] * H for m in OW}
HI_N = H // 128  # 16 contraction tiles

_CACHE = {}


def _build():
    nc = bacc.Bacc("TRN2", target_bir_lowering=False, debug=False, num_devices=NC)

    x_sl = nc.dram_tensor("x_sl", [T, H], F32, kind="ExternalInput")
    cosq = nc.dram_tensor("cosq", [T, HD], F32, kind="ExternalInput")
    sinq = nc.dram_tensor("sinq", [T, HD], F32, kind="ExternalInput")
    cosk = nc.dram_tensor("cosk", [T, HD], F32, kind="ExternalInput")
    sink = nc.dram_tensor("sink", [T, HD], F32, kind="ExternalInput")
    w_sl = {
        "q": nc.dram_tensor("wq_sl", [HSL, OQ], F32, kind="ExternalInput"),
        "k": nc.dram_tensor("wk_sl", [HSL, OK], F32, kind="ExternalInput"),
        "v": nc.dram_tensor("wv_sl", [HSL, OV], F32, kind="ExternalInput"),
        "o": nc.dram_tensor("wo_sl", [HSL, OO], F32, kind="ExternalInput"),
    }
    # mask[p, a, j, f]: a=0 -> pel[k=p, slot j (abs key tile j), q=f] of tile z
    #                  a=1 -> slots 4..7 (abs key tiles 4..7) of tile 7-z
    mask_in = nc.dram_tensor("mask", [128, 2, NJ0, 128], BF16, kind="ExternalInput")
    # cols 0-3: numel for k,v,q,o ; cols 4-7: 1/numel for k,v,q,o
    wconst = nc.dram_tensor("wconst", [1, 8], F32, kind="ExternalInput")
    out = nc.dram_tensor("out", [T, H], F32, kind="ExternalOutput")

    with tile.TileContext(nc) as tc:
        _build_body(nc, tc, x_sl, cosq, sinq, cosk, sink, w_sl, mask_in, wconst, out)

    nc.compile()
    return nc


def _build_body(nc, tc, x_sl, cosq, sinq, cosk, sink, w_sl, mask_in, wconst, out):
    sync = nc.sync

    with (
        tc.tile_pool(name="dram", bufs=1, space="DRAM") as dram,
        tc.tile_pool(name="const", bufs=1) as constp,
        tc.tile_pool(name="vecs", bufs=1) as vecs,
        tc.tile_pool(name="persist", bufs=1) as persist,
    ):
        # ---- DRAM bounce buffers for collectives ----
        wag_kv = dram.tile([HSL, OK + OV], FP8)
        wint_kv = dram.tile([H, OK + OV], FP8, addr_space="Shared")
        wag_q = dram.tile([HSL, OQ], FP8)
        wint_q = dram.tile([H, OQ], FP8, addr_space="Shared")
        wag_o = dram.tile([HSL, OO], FP8)
        wint_o = dram.tile([H, OO], FP8, addr_space="Shared")
        ar_in = dram.tile([1, 8], F32)
        ar_out = dram.tile([1, 8], F32, addr_space="Shared")
        # cols 0:2048 = kT (hk,t); cols 2048:4096 = v (a,hk,d)
        kv_in = dram.tile([128, NKV * T + TB * NKV * HD], BF16)
        kv_out = dram.tile([512, NKV * T + TB * NKV * HD], BF16)

        # ---- constants ----
        ones1 = constp.tile([1, 128], F32)
        nc.vector.memset(ones1, 1.0)
        onescol = constp.tile([128, 1], F32)
        nc.vector.memset(onescol, 1.0)
        wconst_sb = constp.tile([1, 8], F32)
        sync.dma_start(wconst_sb, wconst.ap())
        negrnd = constp.tile([128, 1], F32)
        nc.vector.memset(negrnd, -RND)
        epsb = constp.tile([128, 1], F32)
        nc.vector.memset(epsb, EPS)
        ident = constp.tile([128, 128], F32)
        make_identity(nc, ident)
        identb = constp.tile([128, 128], BF16)
        make_identity(nc, identb)
        mask_sb = constp.tile([128, 2, NJ0, 128], BF16)
        sync.dma_start(mask_sb, mask_in.ap())
        cs = {}
        for nm, t in (("cq", cosq), ("sq", sinq), ("ck", cosk), ("sk", sink)):
            c = constp.tile([128, TB, HD], F32, name=f"cs_{nm}")
            sync.dma_start(c, t.ap().rearrange("(a p) d -> p a d", p=128))
            cs[nm] = c

        # persistent activations (stationary slices contiguous: [.., tb, hi, 128])
        xqT = persist.tile([128, TB, HI_N, 128], BF16)
        qT = persist.tile([128, NH, T], BF16)  # [d, head, t]
        kT = persist.tile([128, NKV, T], BF16)  # [d, kv head, t] (local)
        v_loc = persist.tile([128, TB, NKV, HD], BF16)
        q_sb = persist.tile([128, TB, OQ], F32)
        k_sb = persist.tile([128, TB, OK], F32)
        attn = persist.tile([128, TB, H], F32)
        aT = persist.tile([128, TB, HI_N, 128], BF16)
        # gathered k/v keyed by (group-core z, slot a); key tile j = (zj, aj)
        kT_all = persist.tile([128, NKV, GROUP, TB, 128], BF16)
        v_all = persist.tile([128, GROUP, TB, NKV, 130], BF16)

        # ====== Phase W: x quant/transpose + weight scales + quant + AGs ======
        ptrans_cm = tc.tile_pool(name="ptrans", bufs=2, space="PSUM")
        ptrans = ptrans_cm.__enter__()
        dqx = []
        with (
            tc.tile_pool(name="xraw", bufs=1) as xraw,
            tc.tile_pool(name="wraw_k", bufs=1) as wraw_k,
            tc.tile_pool(name="wraw_v", bufs=1) as wraw_v,
            tc.tile_pool(name="wraw_q", bufs=1) as wraw_q,
            tc.tile_pool(name="wraw_o", bufs=1) as wraw_o,
            tc.tile_pool(name="scr", bufs=1) as scr,
            tc.tile_pool(name="wtmp", bufs=1) as wtmp,
            tc.tile_pool(name="wq8", bufs=2) as wq8,
            tc.tile_pool(name="psmall", bufs=2, space="PSUM") as psmall,
        ):
            # ---- x: load + act_quant + PE transpose ----
            xs_t = []
            for tb in range(TB):
                xs = xraw.tile([128, H], F32, tag="xs", name=f"xs{tb}")
                sync.dma_start(xs, x_sl.ap()[tb * 128 : (tb + 1) * 128, :])
                xs_t.append(xs)

            wraws = {"k": wraw_k, "v": wraw_v, "q": wraw_q, "o": wraw_o}
            worder = ("k", "v", "q", "o")
            wab = {}
            for m in worder:
                for pt in range(2):
                    wr = wraws[m].tile([128, OW[m]], F32, name=f"wr_{m}{pt}")
                    sync.dma_start(wr, w_sl[m].ap()[pt * 128 : (pt + 1) * 128, :])
                    wab[(m, pt)] = wr

            for tb in range(TB):
                xs = xs_t[tb]
                axm = vecs.tile([128, 1], F32, name=f"axm{tb}")
                nc.vector.tensor_reduce(
                    axm, xs, axis=AX, op=OP.max, apply_absolute_value=True
                )
                rsx = vecs.tile([128, 1], F32, name=f"rsx{tb}")
                nc.vector.reciprocal(rsx, axm)
                sxq = vecs.tile([128, 1], F32, name=f"sxq{tb}")
                nc.vector.tensor_scalar_mul(sxq, rsx, 127.0)
                dq = vecs.tile([128, 1], F32, name=f"dqx{tb}")
                nc.vector.tensor_scalar_mul(dq, axm, 1.0 / 127.0)
                dqx.append(dq)
                nc.vector.tensor_scalar(
                    xs, xs, sxq, RND, op0=OP.mult, op1=OP.add
                )
                xqb = xraw.tile([128, H], BF16, tag="xqb", name=f"xqb{tb}")
                nc.vector.tensor_scalar_add(xqb, xs, -RND)
                for hg in range(0, HI_N, 4):
                    pt4 = ptrans.tile(
                        [128, 4, 128], BF16, tag="ptr",
                        padded_shape=[128, 4, 256], name="pt4x",
                    )
                    for i in range(4):
                        hi = hg + i
                        nc.tensor.transpose(
                            pt4[:, i, :], xqb[:, hi * 128 : (hi + 1) * 128], identb
                        )
                    nc.scalar.activation(
                        xqT[:, tb, hg : hg + 4, :], pt4, AF.Copy
                    )

            # ---- weight abs-sums on the scalar engine (accumulate output) ----
            red0 = vecs.tile([128, 4], F32, name="red0")
            red1 = vecs.tile([128, 4], F32, name="red1")
            for mi, m in enumerate(worder):
                for pt, red in ((0, red0), (1, red1)):
                    sc = scr.tile([128, OW[m]], F32, tag="scr", name=f"sc_{m}{pt}")
                    nc.scalar.activation(
                        sc, wab[(m, pt)], AF.Abs, accum_out=red[:, mi : mi + 1]
                    )
            redc = vecs.tile([128, 4], F32, name="redc")
            nc.vector.tensor_add(redc, red0, red1)
            ps = psmall.tile([1, 4], F32, name="ps_sums", tag="psm")
            nc.tensor.matmul(ps, onescol, redc, start=True, stop=True)
            sums = vecs.tile([1, 8], F32, name="sums")
            nc.vector.memset(sums, 0.0)
            nc.scalar.copy(sums[:, 0:4], ps)
            nc.scalar.dma_start(ar_in, sums)
            nc.gpsimd.collective_compute(
                "AllReduce", OP.add, replica_groups=[list(range(NC))],
                ins=[ar_in.opt()], outs=[ar_out.opt()],
            )

            # ---- scales from the AllReduce ----
            g = vecs.tile([1, 8], F32, name="g")
            nc.scalar.dma_start(g, ar_out)
            r4 = vecs.tile([1, 4], F32, name="r4")
            nc.vector.reciprocal(r4, g[:, 0:4])
            sw8 = vecs.tile([1, 8], F32, name="sw8")
            nc.vector.tensor_mul(sw8[:, 0:4], r4, wconst_sb[:, 0:4])
            nc.vector.tensor_mul(sw8[:, 4:8], g[:, 0:4], wconst_sb[:, 4:8])
            pb = psmall.tile([128, 8], F32, name="pb", tag="psm")
            nc.tensor.matmul(pb, ones1, sw8, start=True, stop=True)
            sb8 = vecs.tile([128, 8], F32, name="sb8")
            nc.scalar.copy(sb8, pb)
            rswb = {m: sb8[:, 4 + mi : 5 + mi] for mi, m in enumerate(worder)}

            def w_quant(m, mi, dst, col0, eng=None):
                eng = eng or nc.vector
                for pt in range(2):
                    wr = wab[(m, pt)]
                    t1 = wtmp.tile(
                        [128, OW[m]], F32, tag=f"wtmp{mi % 2}", name=f"t1_{m}{pt}"
                    )
                    eng.tensor_scalar(
                        t1, wr, sb8[:, mi : mi + 1], RND, op0=OP.mult, op1=OP.add
                    )
                    nc.scalar.activation(t1, t1, AF.Identity, bias=negrnd)
                    wi = wq8.tile([128, OW[m]], FP8, tag="wi")
                    eng.tensor_scalar(wi, t1, 1.0, -1.0, op0=OP.min, op1=OP.max)
                    nc.scalar.dma_start(
                        dst[pt * 128 : (pt + 1) * 128, col0 : col0 + OW[m]], wi
                    )

            w_quant("k", 0, wag_kv, 0, nc.vector)
            w_quant("v", 1, wag_kv, OK, nc.gpsimd)
            nc.gpsimd.collective_compute(
                "AllGather", OP.bypass, replica_groups=[list(range(NC))],
                ins=[wag_kv.opt()], outs=[wint_kv.opt()],
            )
            w_quant("q", 2, wag_q, 0)
            nc.gpsimd.collective_compute(
                "AllGather", OP.bypass, replica_groups=[list(range(NC))],
                ins=[wag_q.opt()], outs=[wint_q.opt()],
            )
            w_quant("o", 3, wag_o, 0)

        # dequant vectors (absmax/127 * 1/s_w)
        dqv = {}
        for m in ("q", "k", "v", "o"):
            for tb in range(TB):
                d = vecs.tile([128, 1], F32, name=f"dqv_{m}{tb}")
                nc.vector.tensor_mul(d, dqx[tb], rswb[m])
                dqv[(m, tb)] = d

        def rope_batch(src_sb, tb, nh, cosn, sinn, dstT, ropep, label):
            w = nh * 128
            blk = src_sb[:, tb, :]  # [128, w] f32
            sq = ropep.tile([128, w], F32, tag="unf", padded_shape=[128, NH * 128])
            nc.scalar.activation(sq, blk, AF.Square)
            ms = vecs.tile([128, nh], F32, name=f"ms_{label}{tb}")
            nc.vector.tensor_reduce(
                ms, sq.rearrange("p (h d) -> p h d", h=nh), axis=AX, op=OP.add
            )
            rms = vecs.tile([128, nh], F32, name=f"rms_{label}{tb}")
            nc.scalar.activation(rms, ms, AF.Sqrt, scale=1.0 / HD, bias=epsb)
            rn = vecs.tile([128, nh], F32, name=f"rn_{label}{tb}")
            nc.vector.reciprocal(rn, rms)
            rnb = rn.to_broadcast([128, nh, 128])
            blk3 = blk.rearrange("p (h d) -> p h d", h=nh)
            un2 = ropep.tile(
                [128, nh * 128], F32, tag="unf", padded_shape=[128, NH * 128],
                name="un2",
            )
            un = un2.rearrange("p (h d) -> p h d", h=nh)
            nc.vector.tensor_mul(un, blk3, rnb)
            cosb = (
                cs[cosn][:, tb, :]
                .rearrange("p (one d) -> p one d", one=1)
                .to_broadcast([128, nh, 128])
            )
            sinb = (
                cs[sinn][:, tb, :]
                .rearrange("p (one d) -> p one d", one=1)
                .to_broadcast([128, nh, 128])
            )
            ra2 = ropep.tile([128, nh * 128], F32, tag="ra", padded_shape=[128, NH * 128])
            ra = ra2.rearrange("p (h d) -> p h d", h=nh)
            nc.vector.tensor_mul(ra, un, cosb)
            rb2 = ropep.tile([128, nh * 128], F32, tag="rb", padded_shape=[128, NH * 128])
            rb = rb2.rearrange("p (h d) -> p h d", h=nh)
            nc.vector.tensor_mul(rb[:, :, 0:64], un[:, :, 64:128], sinb[:, :, 0:64])
            nc.vector.tensor_mul(rb[:, :, 64:128], un[:, :, 0:64], sinb[:, :, 64:128])
            raf = ropep.tile(
                [128, nh * 128], BF16, tag="raf", padded_shape=[128, NH * 128]
            )
            nc.vector.tensor_add(raf, ra2, rb2)
            for hg in range(0, nh, 4):
                pt4 = ptrans.tile(
                    [128, 4, 128], BF16, tag="ptr", padded_shape=[128, 4, 256]
                )
                for i in range(4):
                    nc.tensor.transpose(
                        pt4[:, i, :], raf[:, (hg + i) * 128 : (hg + i + 1) * 128],
                        identb,
                    )
                nc.scalar.activation(
                    dstT[:, hg : hg + 4, tb * 128 : (tb + 1) * 128], pt4, AF.Copy
                )

        # ====== Phase KV/Q: projections (hi-outer, chunk-inner) ======
        def proj_load(wint_src, o_w, m, wpool):
            src3 = wint_src.rearrange("(hi p) o -> p hi o", p=128)
            chunks = []
            for cg in range(4):
                wst = wpool.tile(
                    [128, 4, o_w], FP8, tag="wst",
                    padded_shape=[128, 4, OK + OV], name=f"wst_{m}{cg}",
                )
                sync.dma_start(wst, src3[:, cg * 4 : (cg + 1) * 4, :])
                chunks.append(wst)
            return chunks

        def proj_tb(chunks, o_w, tb, ppool, evac):
            """x.T @ w for one token tile; evac(oc, psum_tile) per 512-chunk."""
            ncols = o_w // 512
            pps = [
                ppool.tile([128, 512], F32, tag="pp", name=f"pp{tb}_{i}")
                for i in range(ncols)
            ]
            for hi in range(HI_N):
                xsl = xqT[:, tb, hi, :]
                for oc in range(ncols):
                    nc.tensor.matmul(
                        pps[oc],
                        xsl,
                        chunks[hi // 4][:, hi % 4, oc * 512 : (oc + 1) * 512],
                        start=(hi == 0),
                        stop=(hi == HI_N - 1),
                    )
            for oc in range(ncols):
                evac(oc, pps[oc])

        with (
            tc.tile_pool(name="wmm1", bufs=5) as wmm1,
            tc.tile_pool(name="pproj", bufs=6, space="PSUM") as pproj,
            tc.tile_pool(name="ropep", bufs=1) as ropep,
        ):
            wst_kv = proj_load(wint_kv, OK + OV, "kv", wmm1)

            def evac_kv(tb):
                def f(oc, pp):
                    if oc < 2:  # k chunks
                        nc.vector.tensor_scalar(
                            k_sb[:, tb, oc * 512 : (oc + 1) * 512],
                            pp, dqv[("k", tb)], None, op0=OP.mult,
                        )
                    else:  # v chunks
                        nc.vector.tensor_scalar(
                            v_loc[:, tb, (oc - 2) * 4 : (oc - 1) * 4, :],
                            pp, dqv[("v", tb)], None, op0=OP.mult,
                        )
                return f

            for tb in range(TB):
                proj_tb(wst_kv, OK + OV, tb, pproj, evac_kv(tb))
                rope_batch(k_sb, tb, NKV, "ck", "sk", kT, ropep, "k")

            # bounce k/v to DRAM and allgather within the batch group
            nc.scalar.dma_start(
                kv_in[:, 0 : NKV * T].rearrange("p (hk t) -> p hk t", hk=NKV), kT
            )
            nc.scalar.dma_start(
                kv_in[:, NKV * T :].rearrange(
                    "p (a hk d) -> p a hk d", a=TB, hk=NKV
                ),
                v_loc,
            )
            cc_kv = nc.gpsimd.collective_compute(
                "AllGather", OP.bypass,
                replica_groups=[[0, 1, 2, 3], [4, 5, 6, 7]],
                ins=[kv_in.opt()], outs=[kv_out.opt()],
            )

            # ---- Q projection + rope (overlaps the KV allgather) ----
            wst_q = proj_load(wint_q, OQ, "q", wmm1)

            def evac_q(tb):
                def f(oc, pp):
                    nc.vector.tensor_scalar(
                        q_sb[:, tb, oc * 512 : (oc + 1) * 512],
                        pp, dqv[("q", tb)], None, op0=OP.mult,
                    )
                return f

            for tb in range(TB):
                proj_tb(wst_q, OQ, tb, pproj, evac_q(tb))
                rope_batch(q_sb, tb, NH, "cq", "sq", qT, ropep, "q")
            cc_o = nc.gpsimd.collective_compute(
                "AllGather", OP.bypass, replica_groups=[list(range(NC))],
                ins=[wag_o.opt()], outs=[wint_o.opt()],
            )
            # keep the o-weight gather off the critical kv-activation gather
            bass._add_dep_helper(
                cc_o.ins, cc_kv.ins, sync=True,
                reason="order wo allgather after kv allgather",
            )
        ptrans_cm.__exit__(None, None, None)

        # ====== Phase A: attention ======
        with tc.tile_pool(name="wmm2", bufs=1) as wmm2:
            # prefetch o_proj weights under the attention phase
            src3o = wint_o.rearrange("(hi p) o -> p hi o", p=128)
            wsto = wmm2.tile([128, HI_N, OO], FP8, tag="wst2")
            sync.dma_start(wsto, src3o)

            # gather readback, keyed by (group-core z, slot a)
            KL = NKV * T  # k part length in kv_out cols
            for zc in range(GROUP):
                rows = kv_out[128 * zc : 128 * zc + 128, :]
                src_k = rows[:, 0:KL].rearrange("d (hk a t) -> d hk a t", hk=NKV, a=TB)
                nc.scalar.dma_start(kT_all[:, :, zc, :, :], src_k)
                src_v = rows[:, KL:].rearrange(
                    "p (a hk d) -> p a hk d", a=TB, hk=NKV
                )
                nc.scalar.dma_start(v_all[:, zc, :, :, 0:128], src_v)
            nc.vector.memset(v_all[:, :, :, :, 128:130], 1.0)
            # key tile j lives at (zj, aj)
            JZA = [(j, 0) if j < GROUP else (7 - j, 1) for j in range(KTILES)]

            with (
                tc.tile_pool(name="pscore", bufs=3, space="PSUM") as pscore,
                tc.tile_pool(name="ppv", bufs=2, space="PSUM") as ppv,
                tc.tile_pool(name="pexp", bufs=4) as pexp,
            ):
                pels = {}

                def scores(h):
                    hk = h // 2
                    pel = pexp.tile([128, KTILES, T], BF16, tag="pel")
                    pels[h] = pel
                    # slots 0..3: both q-tiles (N=256); slots 4..7: q-tile1 only
                    for g in range(2):
                        st = pscore.tile([128, 2, T], F32, tag="st")
                        for i in range(2):
                            zj, aj = JZA[2 * g + i]
                            nc.tensor.matmul(
                                st[:, i, :], kT_all[:, hk, zj, aj, :], qT[:, h, :],
                                start=True, stop=True,
                            )
                        nc.scalar.activation(
                            pel[:, 2 * g : 2 * g + 2, :], st, AF.Exp,
                            scale=INV_SQRT_HD,
                        )
                    st2 = pscore.tile([128, 4, 128], F32, tag="st2")
                    for i in range(4):
                        zj, aj = JZA[4 + i]
                        nc.tensor.matmul(
                            st2[:, i, :],
                            kT_all[:, hk, zj, aj, :], qT[:, h, 128:256],
                            start=True, stop=True,
                        )
                    nc.scalar.activation(
                        pel[:, NJ0:KTILES, 128:256], st2, AF.Exp,
                        scale=INV_SQRT_HD,
                    )
                    # per-core causal masks: q-tile0 all 4 slots; q-tile1 slots 4-7
                    nc.vector.tensor_mul(
                        pel[:, 0:NJ0, 0:128], pel[:, 0:NJ0, 0:128], mask_sb[:, 0, :, :]
                    )
                    nc.vector.tensor_mul(
                        pel[:, NJ0:KTILES, 128:256],
                        pel[:, NJ0:KTILES, 128:256],
                        mask_sb[:, 1, :, :],
                    )

                omax = vecs.tile([128, TB], F32, name="omax")

                def pv(h):
                    hk = h // 2
                    pel = pels.pop(h)
                    for a, nj in ((0, NJ0), (1, NJ1)):
                        po = ppv.tile([128, 132], F32, tag="po")
                        for j in range(nj):
                            zj, aj = JZA[j]
                            nc.tensor.matmul(
                                po[:, 0:129],
                                pel[:, j, a * 128 : (a + 1) * 128],
                                v_all[:, zj, aj, hk, 0:129],
                                start=(j == 0),
                                stop=(j == nj - 1),
                            )
                        rden = vecs.tile([128, 1], F32, name=f"rden{h}_{a}")
                        nc.vector.reciprocal(rden, po[:, 128:129])
                        nc.vector.tensor_scalar(
                            attn[:, a, h * 128 : (h + 1) * 128],
                            po[:, 0:128], rden, None, op0=OP.mult,
                        )
                    # running |attn| max per token tile (feeds o act_quant)
                    oax = vecs.tile([128, TB], F32, name=f"oax{h}")
                    nc.vector.tensor_reduce(
                        oax, attn[:, :, h * 128 : (h + 1) * 128], axis=AX,
                        op=OP.max, apply_absolute_value=True,
                    )
                    if h == 0:
                        nc.vector.tensor_copy(omax, oax)
                    else:
                        nc.vector.tensor_max(omax, omax, oax)

                # software-pipelined: scores two heads ahead of PV
                scores(0)
                scores(1)
                for h in range(NH):
                    if h + 2 < NH:
                        scores(h + 2)
                    pv(h)

            # ====== Phase O: act_quant(attn) + o_proj ======
            with (
                tc.tile_pool(name="oq", bufs=2) as oq,
                tc.tile_pool(name="pproj2", bufs=4, space="PSUM") as pproj2,
                tc.tile_pool(name="ptrans2", bufs=2, space="PSUM") as ptrans2,
                tc.tile_pool(name="osb", bufs=2) as osb,
            ):
                dqo = []
                for tb in range(TB):
                    axm = omax[:, tb : tb + 1]
                    rsx = vecs.tile([128, 1], F32, name=f"orsx{tb}")
                    nc.vector.reciprocal(rsx, axm)
                    sxq = vecs.tile([128, 1], F32, name=f"osxq{tb}")
                    nc.vector.tensor_scalar_mul(sxq, rsx, 127.0)
                    dq = vecs.tile([128, 1], F32, name=f"odqx{tb}")
                    nc.vector.tensor_scalar_mul(dq, axm, 1.0 / 127.0)
                    d2 = vecs.tile([128, 1], F32, name=f"odq2{tb}")
                    nc.vector.tensor_mul(d2, dq, rswb["o"])
                    dqo.append(d2)
                    ar = oq.tile([128, H], F32, tag="ar")
                    nc.vector.tensor_scalar(
                        ar, attn[:, tb, :], sxq, RND, op0=OP.mult, op1=OP.add
                    )
                    aqb = oq.tile([128, H], BF16, tag="aqb")
                    nc.vector.tensor_scalar_add(aqb, ar, -RND)
                    for hg in range(0, HI_N, 4):
                        pt4 = ptrans2.tile([128, 4, 128], BF16, tag="ptr2")
                        for i in range(4):
                            hi = hg + i
                            nc.tensor.transpose(
                                pt4[:, i, :], aqb[:, hi * 128 : (hi + 1) * 128], identb
                            )
                        nc.scalar.activation(
                            aT[:, tb, hg : hg + 4, :], pt4, AF.Copy
                        )

                for tb in range(TB):
                    pps = [
                        pproj2.tile([128, 512], F32, tag="pp2", name=f"pp2_{tb}_{i}")
                        for i in range(4)
                    ]
                    for hi in range(HI_N):
                        asl = aT[:, tb, hi, :]
                        for oc in range(4):
                            nc.tensor.matmul(
                                pps[oc],
                                asl,
                                wsto[:, hi, oc * 512 : (oc + 1) * 512],
                                start=(hi == 0),
                                stop=(hi == HI_N - 1),
                            )
                    for oc in range(4):
                        ot = osb.tile([128, 512], F32, tag="ot")
                        nc.vector.tensor_scalar(ot, pps[oc], dqo[tb], None, op0=OP.mult)
                        sync.dma_start(
                            out.ap()[
                                tb * 128 : (tb + 1) * 128, oc * 512 : (oc + 1) * 512
                            ],
                            ot,
                        )


def _host_inputs(x, cos, sin, wq, wk, wv, wo, qn, kn):
    """Build the 8 per-core input maps (pure slicing / layout transforms)."""
    x2 = np.asarray(x, np.float32).reshape(B * S, H)
    cos = np.asarray(cos, np.float32)
    sin = np.asarray(sin, np.float32)
    qn = np.asarray(qn, np.float32)
    kn = np.asarray(kn, np.float32)
    # fold qk-norm weights into rope tables (exact identity when qn=kn=1)
    qn_rot = np.concatenate([qn[HD // 2 :], qn[: HD // 2]])
    kn_rot = np.concatenate([kn[HD // 2 :], kn[: HD // 2]])
    sgn = np.concatenate(
        [-np.ones(HD // 2, np.float32), np.ones(HD // 2, np.float32)]
    )
    cosq_t = cos * qn[None, :]
    sinq_t = sin * (qn_rot * sgn)[None, :]
    cosk_t = cos * kn[None, :]
    sink_t = sin * (kn_rot * sgn)[None, :]

    wt = {
        "q": np.asarray(wq, np.float32).T,  # [H, OQ]
        "k": np.asarray(wk, np.float32).T,
        "v": np.asarray(wv, np.float32).T,
        "o": np.asarray(wo, np.float32).T,  # [H(=in), OO]
    }
    worder = ("k", "v", "q", "o")
    wconst = np.concatenate(
        [
            np.array([WNUMEL[m] for m in worder], np.float32),
            np.array([1.0 / WNUMEL[m] for m in worder], np.float32),
        ]
    ).reshape(1, 8)

    p = np.arange(128)[:, None]
    f = np.arange(128)[None, :]
    tri = (p <= f)  # pel[k, q] upper-incl triangle within the diagonal tile

    in_maps = []
    for c in range(NC):
        b, z = c // GROUP, c % GROUP
        t0a = b * S + z * 128  # q-tile0 = batch tile z
        t0b = b * S + (7 - z) * 128  # q-tile1 = batch tile 7-z
        rows = np.r_[t0a : t0a + 128, t0b : t0b + 128]
        # masks: [128 k, 2, 4, 128 q]
        mask = np.zeros((128, 2, NJ0, 128), np.float32)
        for j in range(NJ0):  # q-tile0 (tile z) vs key tiles 0..3
            if j < z:
                mask[:, 0, j, :] = 1.0
            elif j == z:
                mask[:, 0, j, :] = tri
        for j in range(NJ0, KTILES):  # q-tile1 (tile 7-z) vs key tiles 4..7
            if j < 7 - z:
                mask[:, 1, j - NJ0, :] = 1.0
            elif j == 7 - z:
                mask[:, 1, j - NJ0, :] = tri
        pos = np.r_[z * 128 : z * 128 + 128, (7 - z) * 128 : (8 - z) * 128]
        m = {
            "x_sl": np.ascontiguousarray(x2[rows]),
            "cosq": np.ascontiguousarray(cosq_t[pos]),
            "sinq": np.ascontiguousarray(sinq_t[pos]),
            "cosk": np.ascontiguousarray(cosk_t[pos]),
            "sink": np.ascontiguousarray(sink_t[pos]),
            "wq_sl": np.ascontiguousarray(wt["q"][c * HSL : (c + 1) * HSL]),
            "wk_sl": np.ascontiguousarray(wt["k"][c * HSL : (c + 1) * HSL]),
            "wv_sl": np.ascontiguousarray(wt["v"][c * HSL : (c + 1) * HSL]),
            "wo_sl": np.ascontiguousarray(wt["o"][c * HSL : (c + 1) * HSL]),
            "mask": mask.astype(ml_dtypes.bfloat16),
            "wconst": wconst,
        }
        in_maps.append(m)
    return in_maps


def kernel(x, cos, sin, wq, wk, wv, wo, qn, kn):
    if "nc" not in _CACHE:
        _CACHE["nc"] = _build()
    nc = _CACHE["nc"]
    in_maps = _host_inputs(x, cos, sin, wq, wk, wv, wo, qn, kn)
    res = bass_utils.run_bass_kernel_spmd(nc, in_maps, core_ids=list(range(NC)))
    full = np.zeros((B * S, H), np.float32)
    for c in range(NC):
        b, z = c // GROUP, c % GROUP
        o = np.asarray(res.results[c]["out"])
        t0a = b * S + z * 128
        t0b = b * S + (7 - z) * 128
        full[t0a : t0a + 128] = o[0:128]
        full[t0b : t0b + 128] = o[128:256]
    return full.reshape(B, S, H)
